# revision 7
# baseline (speedup 1.0000x reference)
"""Trainium2 Bass kernel for nn_AttentionModule_7146825580577.

Strategy: pure data parallel over the batch dim (8192 rows -> 1024 rows
per core, 8 cores), weights replicated.

Device math (per core), feature-transposed layout (features on SBUF
partitions, batch on the free dim), bf16 matmul operands with fp32 PSUM
accumulation:

  - LayerNorms over affine-of-activation inputs use host-side
    column-centered weights, so mean(y) == 0 by construction and only
    sum(y^2) is needed (ones-vector matmul on the PE).
  - seq_len==1 MHA reduces to out_proj(v_proj(kv)); fused on the host
    into single 512x512 matrices; self-attention residual folded as
    I + Wv@Wo.
  - The n2 LayerNorm (post-gating) is folded into the fus_W1 matmul:
    gamma scales fold into weights, the per-sample mean correction is a
    rank-1 matmul term (k=3 packed), betas fold into the bias.
  - 1/sqrt(var) via DVE reciprocal_approx_fast + ACT Sqrt on [1,512]
    stat rows (no PE transposes, no NR loop); istd broadcast across
    partitions on GPSIMD.
  - The two 512-column batch tiles are processed in lockstep per
    (stage, stream) group so each weight chunk is DMA'd from HBM once
    and consumed by both tiles back-to-back; LN chains of group k
    execute under the matmuls of group k+1, keeping the PE dense (and
    the HAM clock-gate warm).
"""
import os
import sys

sys.path.insert(0, "/opt/trn_rl_repo")

import numpy as np
import ml_dtypes

import concourse.bass as bass
import concourse.tile as tile
from concourse import bacc, mybir
from concourse.bass import ts
from concourse.bass_utils import run_bass_kernel_spmd

D = 512
HID = 1024
B = 8192
NCORES = 8
BL = B // NCORES          # rows per core
NBT = BL // D             # batch tiles per core (2)
EPS = 1e-5
F32 = mybir.dt.float32
BF = mybir.dt.bfloat16
FS = [10, 6, 15]          # logit dims per stream
F64 = np.float64
BF_NP = ml_dtypes.bfloat16


# --------------------------------------------------------------------------
# Host-side weight folding (float64)
# --------------------------------------------------------------------------

def _center_cols(W, b):
    W = np.asarray(W, F64)
    b = np.asarray(b, F64)
    return W - W.mean(axis=1, keepdims=True), b - b.mean()


def fold_weights(inp):
    g = lambda k: np.asarray(inp[k], dtype=F64)
    out = {}

    w_hp, b_hp = [], []
    for s in range(3):
        W, b = _center_cols(g("hp_W")[s], g("hp_b")[s])
        w_hp.append(W)
        b_hp.append(b)
    out["w_hp"] = np.stack(w_hp)
    out["b_hp"] = np.stack(b_hp)
    out["g_hp"], out["be_hp"] = g("hp_g"), g("hp_be")

    mhaW, mhab = g("mha_in_W"), g("mha_in_b")
    moW, mob = g("mha_out_W"), g("mha_out_b")
    Wv0, bv0 = mhaW[0][:, 2 * D:], mhab[0][2 * D:]
    Wr, br = _center_cols(np.eye(D) + Wv0 @ moW[0], bv0 @ moW[0] + mob[0])
    out["w_r"], out["b_r"] = Wr, br
    out["g_n1"], out["be_n1"] = g("n1_g"), g("n1_be")

    Wj, bj = [None] * 4, [None] * 4
    for j in (1, 2, 3):
        Wv, bv = mhaW[j][:, 2 * D:], mhab[j][2 * D:]
        Wj[j] = Wv @ moW[j]
        bj[j] = bv @ moW[j] + mob[j]
    # m_verb uses (inst_e, target_e); m_inst (verb, target); m_target (verb, inst)
    mods = [(1, 2), (1, 3), (2, 3)]
    w_m, b_m = [], []
    for s in range(3):
        ja, jb = mods[s]
        w_m.append(np.concatenate([0.5 * Wj[ja], 0.5 * Wj[jb]], axis=0))
        b_m.append(0.5 * (bj[ja] + bj[jb]))
    out["w_m"] = np.stack(w_m)
    out["b_m"] = np.stack(b_m)

    out["w_g"] = g("gate_W")
    out["b_g"] = g("gate_b")

    w_lp, b_lp = [], []
    for s, key in enumerate(["verb", "inst", "target"]):
        W, b = _center_cols(g(f"lp_W_{key}"), g(f"lp_b_{key}"))
        w_lp.append(W)
        b_lp.append(b)
    out["w_lp"] = w_lp
    out["b_lp"] = np.stack(b_lp)
    out["g_lp"], out["be_lp"] = g("lp_g"), g("lp_be")

    W1 = g("fus_W1")
    g2, be2 = g("n2_g"), g("n2_be")
    A1, negc = [], []
    bias_total = g("fus_b1").copy()
    for s in range(3):
        blk = W1[s * D:(s + 1) * D]
        A = g2[s][:, None] * blk
        c = blk.T @ g2[s]
        A1.append(A - A.mean(axis=1, keepdims=True))
        negc.append(-(c - c.mean()))
        bias_total += be2[s] @ blk
    L1 = []
    for s in range(3):
        off = 3 * D + s * (D // 2)
        blk = W1[off: off + D // 2]
        L1.append(blk - blk.mean(axis=1, keepdims=True))
    out["w_f1"] = np.stack(A1)
    out["negc_f1"] = np.stack(negc)
    out["w_f1l"] = np.stack(L1)
    out["b_f1"] = bias_total - bias_total.mean()
    out["g_f1"], out["be_f1"] = g("fus_g1"), g("fus_ge1")

    W2c, b2c = _center_cols(g("fus_W2"), g("fus_b2"))
    out["w_f2"], out["b_f2"] = W2c, b2c
    out["g_f2"], out["be_f2"] = g("fus_g2"), g("fus_ge2")
    return out


def _vec_pp(v, nk):
    """[.., nk*128] feature vector -> per-partition layout [.., 128, nk]."""
    v = np.asarray(v, np.float32)
    return np.ascontiguousarray(v.reshape(v.shape[:-1] + (nk, 128)).swapaxes(-1, -2))


def device_arrays(fw):
    f32 = lambda v: np.ascontiguousarray(np.asarray(v, np.float32))
    bf = lambda v: np.ascontiguousarray(
        np.asarray(v, np.float32).astype(BF_NP))
    dev = {}
    dev["w_hp"] = bf(fw["w_hp"].reshape(3, 8, 128, 512))
    dev["b_hp"] = _vec_pp(fw["b_hp"], 4)
    dev["w_r"] = bf(fw["w_r"].reshape(4, 128, 512))
    dev["b_r"] = _vec_pp(fw["b_r"], 4)
    dev["w_m"] = bf(fw["w_m"].reshape(3, 8, 128, 512))
    dev["b_m"] = _vec_pp(fw["b_m"], 4)
    dev["w_g"] = bf(fw["w_g"].reshape(3, 8, 128, 512))
    for s in range(3):
        dev[f"w_lp{s}"] = bf(fw["w_lp"][s])
    dev["b_lp"] = _vec_pp(fw["b_lp"], 2)
    dev["w_f1"] = bf(fw["w_f1"].reshape(3, 4, 128, 512))
    dev["w_f1l"] = bf(fw["w_f1l"].reshape(3, 2, 128, 512))
    dev["negc_f1"] = bf(fw["negc_f1"][None])     # [1, 3, 512]
    dev["b_f1"] = _vec_pp(fw["b_f1"], 4)
    dev["w_f2"] = bf(fw["w_f2"].reshape(4, 128, 512))
    dev["b_f2"] = _vec_pp(fw["b_f2"], 4)
    for name in ("g_hp", "be_hp", "g_n1", "be_n1", "b_g"):
        dev[name] = _vec_pp(fw[name], 4)
    dev["g_lp"] = _vec_pp(fw["g_lp"], 2)
    dev["be_lp"] = _vec_pp(fw["be_lp"], 2)
    for name in ("g_f1", "be_f1", "g_f2", "be_f2"):
        dev[name] = _vec_pp(fw[name], 4)
    dev["ones_col"] = np.ones((128, 1), BF_NP)
    dev["eps_lhs"] = np.full((1, 1), (D // 2) * EPS, BF_NP)
    dev["one_row"] = np.ones((1, 512), BF_NP)
    return dev


# --------------------------------------------------------------------------
# Device program
# --------------------------------------------------------------------------

def emit_program(tc, io):
    nc = tc.nc
    ACT = mybir.ActivationFunctionType
    ALU = mybir.AluOpType
    from contextlib import ExitStack
    ctx = ExitStack()

    P = lambda name, bufs, space="SBUF": ctx.enter_context(
        tc.tile_pool(name=name, bufs=bufs, space=space))
    const = P("const", 1)
    wpool = P("w", 10)
    xpool = P("x", 8)
    lpool = P("l", 6)
    big = P("big", 15)
    tpool = P("t", 3)
    mp = P("m", 9)
    evp = P("ev", 9)
    sqp = P("sq", 6)
    zp = P("z", 6)
    qp = P("q", 4)
    sgp = P("sg", 4)
    bcp = P("bc", 6)
    op_ = P("o", 4)
    rowf = P("rowf", 6)
    rowi = P("rowi", 3)
    wrp = P("wr", 6)
    ltp = P("lt", 4)
    mm_ps = P("mm_ps", 6, "PSUM")
    st_ps = P("st_ps", 2, "PSUM")

    # ---------------- constants ----------------
    def load(name, shape, rearr=None, dtype=F32):
        t = const.tile(shape, dtype, name=name)
        src = io[name]
        if rearr:
            src = src.rearrange(rearr)
        nc.sync.dma_start(t[:], src)
        return t

    ones_col = load("ones_col", [128, 1], dtype=BF)
    eps_lhs = load("eps_lhs", [1, 1], dtype=BF)
    one_row = load("one_row", [1, 512], dtype=BF)
    negc3 = load("negc_f1", [1, 3, 512], dtype=BF)
    b_hp = load("b_hp", [128, 3, 4], "s p c -> p s c")
    b_r = load("b_r", [128, 4])
    b_m = load("b_m", [128, 3, 4], "s p c -> p s c")
    b_lp = load("b_lp", [128, 3, 2], "s p c -> p s c")
    b_f1 = load("b_f1", [128, 4])
    b_f2 = load("b_f2", [128, 4])
    g_hp = load("g_hp", [128, 3, 4], "s p c -> p s c")
    be_hp = load("be_hp", [128, 3, 4], "s p c -> p s c")
    g_n1 = load("g_n1", [128, 3, 4], "s p c -> p s c")
    be_n1 = load("be_n1", [128, 3, 4], "s p c -> p s c")
    b_g = load("b_g", [128, 3, 4], "s p c -> p s c")
    g_lp = load("g_lp", [128, 3, 2], "s p c -> p s c")
    be_lp = load("be_lp", [128, 3, 2], "s p c -> p s c")
    g_f1 = load("g_f1", [128, 4])
    be_f1 = load("be_f1", [128, 4])
    g_f2 = load("g_f2", [128, 4])
    be_f2 = load("be_f2", [128, 4])
    w_lp = [load(f"w_lp{s}", [FS[s], 256], dtype=BF) for s in range(3)]

    # ---------------- helpers ----------------
    pend = []

    def flush(n=None):
        cnt = len(pend) if n is None else n
        for _ in range(cnt):
            if pend:
                pend.pop(0)()

    def wpair(dram_pair_ap):
        wc = wpool.tile([128, 2, 512], BF, name="wc")
        nc.sync.dma_start(wc[:], dram_pair_ap.rearrange("c p n -> p c n"))
        return wc

    def load_pairs(dram_4d, nk):
        """DMA nk [128,512] weight k-chunks (as nk/2 pair tiles); returns
        per-chunk lhsT accessor fns."""
        fns = []
        for c0 in range(0, nk, 2):
            wc = wpair(dram_4d[c0:c0 + 2])
            for cc in range(2):
                fns.append(lambda m, wc=wc, cc=cc: wc[:, cc, ts(m, 128)])
        return fns

    def emit_mms(lhs_fns, rhs_fn, nm=4):
        ps = [mm_ps.tile([128, 512], F32, name="mm") for _ in range(nm)]
        last = len(lhs_fns) - 1
        for ci, lf in enumerate(lhs_fns):
            rhs = rhs_fn(ci)
            for m in range(nm):
                nc.tensor.matmul(ps[m][:], lf(m), rhs,
                                 start=(ci == 0), stop=(ci == last))
        return ps

    def evict_sq(ps_list, bias_cols, do_sq=True, pool=None, eng="act",
                 sq_eng="dve"):
        ev, sq = [], []
        for c, psx in enumerate(ps_list):
            e = (pool or evp).tile([128, 512], BF, name="evt")
            if eng == "dve":
                nc.vector.tensor_scalar_add(e[:], psx[:], bias_cols[c])
            else:
                nc.scalar.activation(e[:], psx[:], ACT.Identity,
                                     bias=bias_cols[c])
            ev.append(e)
            if do_sq:
                s = sqp.tile([128, 512], BF, name="sqt")
                if sq_eng == "act_ps":
                    nc.scalar.activation(s[:], psx[:], ACT.Square,
                                         bias=bias_cols[c])
                else:
                    nc.vector.tensor_mul(s[:], e[:], e[:])
                sq.append(s)
        return ev, sq

    def stats(sq_list, add_eps=False):
        st = st_ps.tile([1, 512], F32, name="st", tag="stps")
        n = len(sq_list) + (1 if add_eps else 0)
        for c, s in enumerate(sq_list):
            nc.tensor.matmul(st[:], ones_col[:], s[:],
                             start=(c == 0), stop=(c == n - 1))
        if add_eps:
            nc.tensor.matmul(st[:], eps_lhs[:], one_row[:],
                             start=False, stop=True)
        return st

    def istd_chain(st, dim):
        rec = rowf.tile([1, 512], F32, name="rec", tag="rowf")
        nc.vector.reciprocal_approx_fast(rec[:], st[:])
        ist = rowi.tile([1, 512], BF, name="ist", tag="rowi")
        nc.scalar.activation(ist[:], rec[:], ACT.Sqrt, scale=float(dim))
        bc = bcp.tile([128, 512], BF, name="bct")
        nc.gpsimd.partition_broadcast(bc[:], ist[0:1, :])
        return bc

    def ln_finish(ev, st, dim, gam, bet, func, out_tile, via_ts=False):
        bc = istd_chain(st, dim)
        for c, e in enumerate(ev):
            z = zp.tile([128, 512], BF, name="zt")
            nc.vector.tensor_mul(z[:], e[:], bc[:])
            if via_ts:
                nc.vector.tensor_scalar(out_tile[:, c, :], z[:],
                                        gam[:, c:c + 1], bet[:, c:c + 1],
                                        ALU.mult, ALU.add)
            else:
                nc.scalar.activation(out_tile[:, c, :], z[:], func,
                                     bias=bet[:, c:c + 1],
                                     scale=gam[:, c:c + 1])

    # ---------------- per-stage state ----------------
    yh = [[None] * 2 for _ in range(3)]
    e_ = [[None] * 2 for _ in range(3)]
    m_ = [[None] * 2 for _ in range(3)]
    zt = [[None] * 2 for _ in range(3)]
    l_ = [[None] * 2 for _ in range(3)]
    h_ = [None] * 2
    wrow = [[None] * 3 for _ in range(2)]
    hpw = [None] * 3
    mw = [None] * 3
    gw = [None] * 3
    rw = [None]
    f1w = [None]
    f2w = [None]

    def lp_group(s, bt):
        bsl = ts(bt, 512)
        lt = ltp.tile([FS[s], 512], F32, name="ltt", tag="ltt")
        nc.sync.dma_start(lt[:], io[f"lT{s}"][:, bsl])
        lsg = ltp.tile([FS[s], 512], BF, name="lsg", tag="ltt")
        nc.scalar.activation(lsg[:], lt[:], ACT.Sigmoid)
        ps = [mm_ps.tile([128, 512], F32, name="mm") for _ in range(2)]
        for m in range(2):
            nc.tensor.matmul(ps[m][:], w_lp[s][:, ts(m, 128)], lsg[:],
                             start=True, stop=True)
        ev, sq = evict_sq(ps, [b_lp[:, s, c:c + 1] for c in range(2)])

        def fin(s=s, bt=bt, ev=ev, sq=sq):
            st = stats(sq, add_eps=True)
            l_sb = lpool.tile([128, 2, 512], BF, name="l_sb")
            ln_finish(ev, st, D // 2, g_lp[:, s], be_lp[:, s], ACT.Gelu, l_sb)
            l_[s][bt] = l_sb
        pend.append(fin)

    hpx = [[None] * 2 for _ in range(3)]

    def hp_prefetch(s, bt):
        bsl = ts(bt, 512)
        xcs = []
        for c0 in range(0, 8, 2):
            xc = xpool.tile([128, 2, 512], BF, name="xc")
            nc.sync.dma_start(
                xc[:], io[f"xT{s}"][ts(c0 // 2, 256), bsl].rearrange(
                    "(c p) b -> p c b", p=128))
            xcs.append(xc)
        hpx[s][bt] = xcs
        if bt == 0:
            hpw[s] = load_pairs(io["w_hp"][s], 8)

    def hp_group(s, bt):
        xcs = hpx[s][bt]
        ps = emit_mms(hpw[s], lambda c: xcs[c // 2][:, c % 2, :])
        ev, sq = evict_sq(ps, [b_hp[:, s, c:c + 1] for c in range(4)],
                          eng="dve", sq_eng="act_ps")

        def fin(s=s, bt=bt, ev=ev, sq=sq):
            st = stats(sq)
            y_sb = big.tile([128, 4, 512], BF, name="big_sb")
            ln_finish(ev, st, D, g_hp[:, s], be_hp[:, s], ACT.Gelu, y_sb)
            yh[s][bt] = y_sb
        pend.append(fin)

    def r_group(s, bt):
        if rw[0] is None:
            rw[0] = load_pairs(io["w_r"], 4)
        ps = emit_mms(rw[0], lambda c: yh[s][bt][:, c, :])
        ev, sq = evict_sq(ps, [b_r[:, c:c + 1] for c in range(4)],
                          eng="dve")

        def fin(s=s, bt=bt, ev=ev, sq=sq):
            st = stats(sq)
            e_sb = big.tile([128, 4, 512], BF, name="big_sb")
            ln_finish(ev, st, D, g_n1[:, s], be_n1[:, s], ACT.Identity,
                      e_sb, via_ts=True)
            e_[s][bt] = e_sb
        pend.append(fin)

    m_streams = [(1, 2), (0, 2), (0, 1)]

    def m_group(s, bt):
        if bt == 0:
            mw[s] = load_pairs(io["w_m"][s], 8)
        sa, sb = m_streams[s]
        ps = emit_mms(mw[s], lambda c: (e_[sa][bt][:, c, :] if c < 4
                                        else e_[sb][bt][:, c - 4, :]))
        ev, _ = evict_sq(ps, [b_m[:, s, c:c + 1] for c in range(4)],
                         do_sq=False, pool=mp, eng="dve")
        m_[s][bt] = ev

    def g_group(s, bt):
        if bt == 0:
            gw[s] = load_pairs(io["w_g"][s], 8)
        ps = emit_mms(gw[s], lambda c: (e_[s][bt][:, c, :] if c < 4
                                        else m_[s][bt][c - 4][:]))
        t_sb = tpool.tile([128, 4, 512], BF, name="t_sb")
        sqs = []
        for c in range(4):
            sg = sgp.tile([128, 512], BF, name="sgt")
            nc.scalar.activation(sg[:], ps[c][:], ACT.Sigmoid,
                                 bias=b_g[:, s, c:c + 1])
            q = qp.tile([128, 512], BF, name="qt")
            nc.gpsimd.tensor_mul(q[:], sg[:], m_[s][bt][c][:])
            nc.vector.tensor_add(t_sb[:, c, :], e_[s][bt][:, c, :], q[:])
            sqc = sqp.tile([128, 512], BF, name="sqt")
            nc.gpsimd.tensor_mul(sqc[:], t_sb[:, c, :], t_sb[:, c, :])
            sqs.append(sqc)

        def fin(s=s, bt=bt, t_sb=t_sb, sqs=sqs):
            st_sum = st_ps.tile([1, 512], F32, name="st", tag="stps")
            for c in range(4):
                nc.tensor.matmul(st_sum[:], ones_col[:], t_sb[:, c, :],
                                 start=(c == 0), stop=(c == 3))
            st_sq = stats(sqs)
            mu = rowf.tile([1, 512], F32, name="mu", tag="rowf")
            nc.scalar.activation(mu[:], st_sum[:], ACT.Copy, scale=1.0 / D)
            m2 = rowf.tile([1, 512], F32, name="m2", tag="rowf")
            nc.vector.tensor_mul(m2[:], mu[:], mu[:])
            v = rowf.tile([1, 512], F32, name="vv", tag="rowf")
            nc.vector.scalar_tensor_tensor(v[:], m2[:], -float(D), st_sq[:],
                                           ALU.mult, ALU.add)
            rec = rowf.tile([1, 512], F32, name="rec", tag="rowf")
            nc.vector.reciprocal_approx_fast(rec[:], v[:])
            ist = rowi.tile([1, 512], BF, name="ist", tag="rowi")
            nc.scalar.activation(ist[:], rec[:], ACT.Sqrt, scale=float(D))
            wr = wrp.tile([1, 512], BF, name="wr1")
            nc.vector.tensor_mul(wr[:], mu[:], ist[:])
            wrow[bt][s] = wr
            bc = bcp.tile([128, 512], BF, name="bct")
            nc.gpsimd.partition_broadcast(bc[:], ist[0:1, :])
            zt_sb = big.tile([128, 4, 512], BF, name="big_sb")
            for c in range(4):
                nc.gpsimd.tensor_mul(zt_sb[:, c, :], t_sb[:, c, :], bc[:])
            zt[s][bt] = zt_sb
        pend.append(fin)

    def f1_group(bt):
        if bt == 0:
            f1lw = [load_pairs(io["w_f1l"][s], 2) for s in range(3)]
            f1ww = [load_pairs(io["w_f1"][s], 4) for s in range(3)]
            f1w[0] = (f1lw, f1ww)
        f1lw, f1ww = f1w[0]
        ps = [mm_ps.tile([128, 512], F32, name="mm") for _ in range(4)]
        seq = []
        for s in range(3):
            seq += [(f1lw[s][c], l_[s][bt][:, c, :]) for c in range(2)]
        for s in (2, 1, 0):
            seq += [(f1ww[s][c], zt[s][bt][:, c, :]) for c in range(4)]
        for s in (2, 1, 0):
            seq.append((lambda m, s=s: negc3[0:1, s, ts(m, 128)],
                        wrow[bt][s][:]))
        last = len(seq) - 1
        for ci, (lf, rhs) in enumerate(seq):
            for m in range(4):
                nc.tensor.matmul(ps[m][:], lf(m), rhs,
                                 start=(ci == 0), stop=(ci == last))
        ev, sq = evict_sq(ps, [b_f1[:, c:c + 1] for c in range(4)])

        def fin(bt=bt, ev=ev, sq=sq):
            st = stats(sq)
            h_sb = big.tile([128, 4, 512], BF, name="big_sb")
            ln_finish(ev, st, D, g_f1, be_f1, ACT.Gelu, h_sb)
            h_[bt] = h_sb
        pend.append(fin)

    def f2_group(bt):
        bsl = ts(bt, 512)
        if bt == 0:
            f2w[0] = load_pairs(io["w_f2"], 4)
        ps = emit_mms(f2w[0], lambda c: h_[bt][:, c, :])
        ev, sq = evict_sq(ps, [b_f2[:, c:c + 1] for c in range(4)])

        def fin(bt=bt, bsl=bsl, ev=ev, sq=sq):
            st = stats(sq)
            bc = istd_chain(st, D)
            for c, et in enumerate(ev):
                z = op_.tile([128, 512], F32, name="ot")
                nc.vector.tensor_mul(z[:], et[:], bc[:])
                o = op_.tile([128, 512], F32, name="ot")
                nc.vector.tensor_scalar(o[:], z[:], g_f2[:, c:c + 1],
                                        be_f2[:, c:c + 1], ALU.mult, ALU.add)
                nc.sync.dma_start(io["outT"][ts(c, 128), bsl], o[:])
        pend.append(fin)

    # ---------------- emission schedule ----------------
    hp_prefetch(0, 0); hp_prefetch(0, 1)
    for s in range(3):
        for bt in range(2):
            lp_group(s, bt)
    hp_prefetch(1, 0); hp_prefetch(1, 1)
    hp_group(0, 0); hp_group(0, 1)
    flush(6)                      # lp fins (under hp0 matmuls)
    hp_group(1, 0); hp_group(1, 1)
    hp_prefetch(2, 0); hp_prefetch(2, 1)
    flush(2)                      # hp0 fins
    hp_group(2, 0); hp_group(2, 1)
    flush(2)                      # hp1 fins
    r_group(0, 0); r_group(0, 1)
    flush(2)                      # hp2 fins
    r_group(1, 0); r_group(1, 1)
    flush(2)                      # r0 fins -> e0
    r_group(2, 0); r_group(2, 1)
    flush(2)                      # r1 fins -> e1
    m_group(2, 0); m_group(2, 1)  # needs e0, e1
    flush(2)                      # r2 fins -> e2
    g_group(2, 0); g_group(2, 1)
    m_group(1, 0); m_group(1, 1)  # needs e0, e2
    flush(2)                      # gate2 fins -> zt2 (under m1 matmuls)
    g_group(1, 0); g_group(1, 1)
    m_group(0, 0); m_group(0, 1)  # needs e1, e2
    flush(2)                      # gate1 fins -> zt1
    g_group(0, 0); g_group(0, 1)
    flush(2)                      # gate0 fins -> zt0
    f1_group(0)
    f1_group(1)
    flush(1)                      # f1_0 fin -> h0 (under f1_1 matmuls)
    f2_group(0)
    flush(1)                      # f1_1 fin -> h1
    f2_group(1)
    flush()                       # f2 fins -> out DMAs
    ctx.close()


def build_program():
    nc = bacc.Bacc("TRN2", target_bir_lowering=False, debug=False,
                   num_devices=NCORES)
    io = {}

    def din(name, shape, dtype=F32):
        io[name] = nc.dram_tensor(name, list(shape), dtype,
                                  kind="ExternalInput").ap()

    for s in range(3):
        din(f"xT{s}", (HID, BL), dtype=BF)
        din(f"lT{s}", (FS[s], BL))
    din("w_hp", (3, 8, 128, 512), dtype=BF)
    din("b_hp", (3, 128, 4))
    din("w_r", (4, 128, 512), dtype=BF)
    din("b_r", (128, 4))
    din("w_m", (3, 8, 128, 512), dtype=BF)
    din("b_m", (3, 128, 4))
    din("w_g", (3, 8, 128, 512), dtype=BF)
    for s in range(3):
        din(f"w_lp{s}", (FS[s], 256), dtype=BF)
    din("b_lp", (3, 128, 2))
    din("w_f1", (3, 4, 128, 512), dtype=BF)
    din("w_f1l", (3, 2, 128, 512), dtype=BF)
    din("negc_f1", (1, 3, 512), dtype=BF)
    din("b_f1", (128, 4))
    din("w_f2", (4, 128, 512), dtype=BF)
    din("b_f2", (128, 4))
    for name in ("g_hp", "be_hp", "g_n1", "be_n1", "b_g"):
        din(name, (3, 128, 4))
    for name in ("g_lp", "be_lp"):
        din(name, (3, 128, 2))
    for name in ("g_f1", "be_f1", "g_f2", "be_f2"):
        din(name, (128, 4))
    din("ones_col", (128, 1), dtype=BF)
    din("eps_lhs", (1, 1), dtype=BF)
    din("one_row", (1, 512), dtype=BF)
    io["outT"] = nc.dram_tensor("outT", [D, BL], F32,
                                kind="ExternalOutput").ap()

    with tile.TileContext(nc) as tc:
        emit_program(tc, io)
    nc.compile()
    return nc


def make_in_maps(inputs):
    fw = fold_weights(inputs)
    dev = device_arrays(fw)
    hidden = [np.asarray(inputs["verb_hidden"], np.float32),
              np.asarray(inputs["inst_hidden"], np.float32),
              np.asarray(inputs["target_hidden"], np.float32)]
    logits = [np.asarray(inputs["verb_logits"], np.float32),
              np.asarray(inputs["inst_logits"], np.float32),
              np.asarray(inputs["target_logits"], np.float32)]
    in_maps = []
    for core in range(NCORES):
        rows = slice(core * BL, (core + 1) * BL)
        m = dict(dev)
        for s in range(3):
            m[f"xT{s}"] = np.ascontiguousarray(
                hidden[s][rows].T.astype(BF_NP))
            m[f"lT{s}"] = np.ascontiguousarray(logits[s][rows].T)
        in_maps.append(m)
    return in_maps


_NC_CACHE = None


def _run(inputs, **spmd_kwargs):
    global _NC_CACHE
    if _NC_CACHE is None:
        _NC_CACHE = build_program()
    nc = _NC_CACHE
    in_maps = make_in_maps(inputs)
    res = run_bass_kernel_spmd(nc, in_maps, list(range(NCORES)),
                               **spmd_kwargs)
    out = np.empty((B, D), dtype=np.float32)
    for core in range(NCORES):
        out[core * BL:(core + 1) * BL] = res.results[core]["outT"].T
    return out, res


def kernel(**inputs) -> np.ndarray:
    return _run(inputs)[0]


def kernel_profiled(inputs, tmpdir=None):
    """Returns (out, BassKernelResults) with an NTFF-based profile."""
    return _run(inputs, trace=True, tmpdir=tmpdir)


# revision 32
# speedup vs baseline: 1.3159x; 1.3159x over previous
"""Trainium2 Bass kernel for nn_AttentionModule_7146825580577.

Strategy: pure data parallel over the batch dim (8192 rows -> 1024 rows
per core, 8 cores), weights replicated.

Device math (per core), feature-transposed layout (features on SBUF
partitions, batch on the free dim), bf16 matmul operands with fp32 PSUM
accumulation:

  - LayerNorms over affine-of-activation inputs use host-side
    column-centered weights, so mean(y) == 0 by construction and only
    sum(y^2) is needed (ones-vector matmul on the PE).
  - seq_len==1 MHA reduces to out_proj(v_proj(kv)); fused on the host
    into single 512x512 matrices; self-attention residual folded as
    I + Wv@Wo.
  - The n2 LayerNorm (post-gating) is folded into the fus_W1 matmul:
    gamma scales fold into weights, the per-sample mean correction is a
    rank-1 matmul term (k=3 packed), betas fold into the bias.
  - 1/sqrt(var) via DVE reciprocal_approx_fast + ACT Sqrt on [1,512]
    stat rows (no PE transposes, no NR loop); istd broadcast across
    partitions on GPSIMD.
  - The two 512-column batch tiles are processed in lockstep per
    (stage, stream) group so each weight chunk is DMA'd from HBM once
    and consumed by both tiles back-to-back; LN chains of group k
    execute under the matmuls of group k+1, keeping the PE dense (and
    the HAM clock-gate warm).
"""
import os
import sys

sys.path.insert(0, "/opt/trn_rl_repo")

import numpy as np
import ml_dtypes

import concourse.bass as bass
import concourse.tile as tile
from concourse import bacc, mybir
from concourse.bass import ts
from concourse.bass_utils import run_bass_kernel_spmd

D = 512
HID = 1024
B = 8192
NCORES = 8
BL = B // NCORES          # rows per core
NBT = BL // D             # batch tiles per core (2)
EPS = 1e-5
F32 = mybir.dt.float32
BF = mybir.dt.bfloat16
FS = [10, 6, 15]          # logit dims per stream
F64 = np.float64
BF_NP = ml_dtypes.bfloat16


# --------------------------------------------------------------------------
# Host-side weight folding (float64)
# --------------------------------------------------------------------------

def _center_cols(W, b):
    W = np.asarray(W, F64)
    b = np.asarray(b, F64)
    return W - W.mean(axis=1, keepdims=True), b - b.mean()


def fold_weights(inp):
    g = lambda k: np.asarray(inp[k], dtype=F64)
    out = {}

    w_hp, b_hp = [], []
    for s in range(3):
        W, b = _center_cols(g("hp_W")[s], g("hp_b")[s])
        w_hp.append(W)
        b_hp.append(b)
    out["w_hp"] = np.stack(w_hp)
    out["b_hp"] = np.stack(b_hp)
    out["g_hp"], out["be_hp"] = g("hp_g"), g("hp_be")

    mhaW, mhab = g("mha_in_W"), g("mha_in_b")
    moW, mob = g("mha_out_W"), g("mha_out_b")
    Wv0, bv0 = mhaW[0][:, 2 * D:], mhab[0][2 * D:]
    Wr, br = _center_cols(np.eye(D) + Wv0 @ moW[0], bv0 @ moW[0] + mob[0])
    out["w_r"], out["b_r"] = Wr, br
    out["g_n1"], out["be_n1"] = g("n1_g"), g("n1_be")

    Wj, bj = [None] * 4, [None] * 4
    for j in (1, 2, 3):
        Wv, bv = mhaW[j][:, 2 * D:], mhab[j][2 * D:]
        Wj[j] = Wv @ moW[j]
        bj[j] = bv @ moW[j] + mob[j]
    # m_verb uses (inst_e, target_e); m_inst (verb, target); m_target (verb, inst)
    mods = [(1, 2), (1, 3), (2, 3)]
    streams = [(1, 2), (0, 2), (0, 1)]
    be1 = g("n1_be")
    w_m, b_m = [], []
    for s in range(3):
        ja, jb = mods[s]
        sa, sb = streams[s]
        w_m.append(np.concatenate([0.5 * Wj[ja], 0.5 * Wj[jb]], axis=0))
        # device e-tiles carry only g*z (be_n1 folded here)
        b_m.append(0.5 * (bj[ja] + bj[jb])
                   + 0.5 * (be1[sa] @ Wj[ja] + be1[sb] @ Wj[jb]))
    out["w_m"] = np.stack(w_m)
    out["b_m"] = np.stack(b_m)

    gW = g("gate_W")
    out["w_g"] = gW
    out["b_g"] = g("gate_b") + np.stack(
        [be1[s] @ gW[s][:D] for s in range(3)])

    w_lp, b_lp = [], []
    for s, key in enumerate(["verb", "inst", "target"]):
        W, b = _center_cols(g(f"lp_W_{key}"), g(f"lp_b_{key}"))
        w_lp.append(W)
        b_lp.append(b)
    out["w_lp"] = w_lp
    out["b_lp"] = np.stack(b_lp)
    out["g_lp"], out["be_lp"] = g("lp_g"), g("lp_be")

    W1 = g("fus_W1")
    g2, be2 = g("n2_g"), g("n2_be")
    A1, negc = [], []
    bias_total = g("fus_b1").copy()
    for s in range(3):
        blk = W1[s * D:(s + 1) * D]
        A = g2[s][:, None] * blk
        c = blk.T @ g2[s]
        A1.append(A - A.mean(axis=1, keepdims=True))
        negc.append(-(c - c.mean()))
        bias_total += be2[s] @ blk
    L1 = []
    for s in range(3):
        off = 3 * D + s * (D // 2)
        blk = W1[off: off + D // 2]
        L1.append(blk - blk.mean(axis=1, keepdims=True))
    out["w_f1"] = np.stack(A1)
    out["negc_f1"] = np.stack(negc)
    out["w_f1l"] = np.stack(L1)
    out["b_f1"] = bias_total - bias_total.mean()
    out["g_f1"], out["be_f1"] = g("fus_g1"), g("fus_ge1")

    W2c, b2c = _center_cols(g("fus_W2"), g("fus_b2"))
    out["w_f2"], out["b_f2"] = W2c, b2c
    out["g_f2"], out["be_f2"] = g("fus_g2"), g("fus_ge2")
    return out


def _vec_pp(v, nk):
    """[.., nk*128] feature vector -> per-partition layout [.., 128, nk]."""
    v = np.asarray(v, np.float32)
    return np.ascontiguousarray(v.reshape(v.shape[:-1] + (nk, 128)).swapaxes(-1, -2))


CVEC_SPEC = [("b_hp", 4, 3), ("b_m", 4, 3), ("g_hp", 4, 3),
             ("be_hp", 4, 3), ("g_n1", 4, 3), ("be_n1", 4, 3),
             ("b_g", 4, 3), ("b_r", 4, 1), ("b_f1", 4, 1),
             ("b_f2", 4, 1), ("g_f1", 4, 1), ("be_f1", 4, 1),
             ("g_f2", 4, 1), ("be_f2", 4, 1), ("b_lp", 2, 3),
             ("g_lp", 2, 3), ("be_lp", 2, 3)]
CVEC_OFF = {}
_off = 0
for _n, _k, _s in CVEC_SPEC:
    CVEC_OFF[_n] = (_off, _k * _s)
    _off += _k * _s
CVEC_NCOLS = _off


def device_arrays(fw):
    f32 = lambda v: np.ascontiguousarray(np.asarray(v, np.float32))
    bf = lambda v: np.ascontiguousarray(
        np.asarray(v, np.float32).astype(BF_NP))
    def pairs(W, nk):
        # [nk*128, 512] -> [128, nk/2, 2, 512]: whole stage in ONE DMA,
        # contiguous nk*1KB run per partition.
        W = np.asarray(W, np.float32)
        return bf(W.reshape(nk // 2, 2, 128, 512).transpose(2, 0, 1, 3))
    dev = {}
    dev["w_hp"] = bf(np.stack([pairs(fw["w_hp"][s], 8) for s in range(3)]))
    dev["w_r"] = pairs(fw["w_r"], 4)
    dev["w_m"] = bf(np.stack([pairs(fw["w_m"][s], 8) for s in range(3)]))
    dev["w_g"] = bf(np.stack([pairs(fw["w_g"][s], 8) for s in range(3)]))
    for s in range(3):
        dev[f"w_lp{s}"] = bf(fw["w_lp"][s])
    dev["w_f1all"] = bf(np.concatenate(
        [pairs(fw["w_f1l"][s], 2) for s in range(3)]
        + [pairs(fw["w_f1"][s], 4) for s in range(3)], axis=1))
    dev["negc_f1"] = bf(fw["negc_f1"][None])     # [1, 3, 512]
    dev["w_f2"] = pairs(fw["w_f2"], 4)
    # all per-partition bias/gamma vectors in one [128, ncols] tensor
    cols = []
    for name, nk, _ns in CVEC_SPEC:
        v = _vec_pp(fw[name], nk)
        v = v.reshape(128, -1) if v.ndim == 2 else \
            np.ascontiguousarray(v.transpose(1, 0, 2)).reshape(128, -1)
        assert v.shape[1] == CVEC_OFF[name][1], name
        cols.append(v)
    dev["cvec"] = np.ascontiguousarray(np.concatenate(cols, axis=1))
    dev["ones_col"] = np.ones((128, 1), BF_NP)
    dev["eps_lhs"] = np.full((1, 1), (D // 2) * EPS, BF_NP)
    dev["one_row"] = np.ones((1, 512), BF_NP)
    return dev


# --------------------------------------------------------------------------
# Device program
# --------------------------------------------------------------------------

def emit_program(tc, io):
    nc = tc.nc
    ACT = mybir.ActivationFunctionType
    ALU = mybir.AluOpType
    from contextlib import ExitStack
    ctx = ExitStack()

    P = lambda name, bufs, space="SBUF": ctx.enter_context(
        tc.tile_pool(name=name, bufs=bufs, space=space))
    const = P("const", 1)
    wpool = P("w", 3)
    xpool = P("x", 2)
    lpool = P("l", 6)
    big = P("big", 12)
    tpool = P("t", 2)
    mp = P("m", 8)
    evp = P("ev", 8)
    sqp = P("sq", 5)
    zp = P("z", 6)
    qp = P("q", 4)
    sgp = P("sg", 4)
    bcp = P("bc", 4)
    op_ = P("o", 4)
    rowf = P("rowf", 3)
    rowi = P("rowi", 3)
    wrp = P("wr", 6)
    ltp = P("lt", 4)
    f1p = P("f1w", 1)
    mm_ps = P("mm_ps", 6, "PSUM")
    st_ps = P("st_ps", 2, "PSUM")

    # ---------------- constants ----------------
    def load(name, shape, rearr=None, dtype=F32):
        t = const.tile(shape, dtype, name=name)
        src = io[name]
        if rearr:
            src = src.rearrange(rearr)
        nc.sync.dma_start(t[:], src)
        return t

    ones_col = load("ones_col", [128, 1], dtype=BF)
    eps_lhs = load("eps_lhs", [1, 1], dtype=BF)
    one_row = load("one_row", [1, 512], dtype=BF)
    negc3 = load("negc_f1", [1, 3, 512], dtype=BF)
    cvec = load("cvec", [128, CVEC_NCOLS])

    def cv(name):
        off, n = CVEC_OFF[name]
        ap = cvec[:, off:off + n]
        if n > 4:
            ap = ap.rearrange("p (s c) -> p s c", s=3)
        return ap

    b_hp, b_m, g_hp, be_hp = cv("b_hp"), cv("b_m"), cv("g_hp"), cv("be_hp")
    g_n1, be_n1, b_g = cv("g_n1"), cv("be_n1"), cv("b_g")
    b_r, b_f1, b_f2 = cv("b_r"), cv("b_f1"), cv("b_f2")
    g_f1, be_f1, g_f2, be_f2 = cv("g_f1"), cv("be_f1"), cv("g_f2"), cv("be_f2")
    b_lp, g_lp, be_lp = cv("b_lp"), cv("g_lp"), cv("be_lp")
    w_lp = [load(f"w_lp{s}", [FS[s], 256], dtype=BF) for s in range(3)]

    # ---------------- helpers ----------------
    pend = []

    def flush(n=None):
        cnt = len(pend) if n is None else n
        for _ in range(cnt):
            if pend:
                pend.pop(0)()

    def load_pairs(dram_stage, npairs):
        """One DMA for a whole [128, npairs, 2, 512] weight stage."""
        wc = wpool.tile([128, npairs, 2, 512], BF, name="wc", tag="wc")
        nc.sync.dma_start(wc[:], dram_stage)
        fns = []
        for i in range(npairs):
            for cc in range(2):
                fns.append(lambda m, wc=wc, i=i, cc=cc:
                           wc[:, i, cc, ts(m, 128)])
        return fns

    def emit_mms(lhs_fns, rhs_fn, nm=4):
        ps = [mm_ps.tile([128, 512], F32, name="mm") for _ in range(nm)]
        last = len(lhs_fns) - 1
        for ci, lf in enumerate(lhs_fns):
            rhs = rhs_fn(ci)
            for m in range(nm):
                nc.tensor.matmul(ps[m][:], lf(m), rhs,
                                 start=(ci == 0), stop=(ci == last))
        return ps

    def evict_sq(ps_list, bias_cols, do_sq=True, pool=None, dve_half=True,
                 sq_ps=False):
        """Evict psum chunks (+bias) to bf16 SBUF; optionally square them.
        Evictions alternate ACT/DVE so the last chunk lands fast. With
        sq_ps, squares come straight from PSUM on ACT (Square is in every
        activation table, so no table reload)."""
        ev, sq = [], []
        for c, psx in enumerate(ps_list):
            e = (pool or evp).tile([128, 512], BF, name="evt")
            if dve_half and c % 2 == 1:
                nc.vector.tensor_scalar_add(e[:], psx[:], bias_cols[c])
            else:
                nc.scalar.activation(e[:], psx[:], ACT.Identity,
                                     bias=bias_cols[c])
            ev.append(e)
            if do_sq:
                s = sqp.tile([128, 512], BF, name="sqt")
                if sq_ps:
                    nc.scalar.activation(s[:], psx[:], ACT.Square,
                                         bias=bias_cols[c])
                else:
                    nc.vector.tensor_mul(s[:], e[:], e[:])
                sq.append(s)
        return ev, sq

    def stats(sq_list, add_eps=False):
        st = st_ps.tile([1, 512], F32, name="st", tag="stps")
        n = len(sq_list) + (1 if add_eps else 0)
        for c, s in enumerate(sq_list):
            nc.tensor.matmul(st[:], ones_col[:], s[:],
                             start=(c == 0), stop=(c == n - 1))
        if add_eps:
            nc.tensor.matmul(st[:], eps_lhs[:], one_row[:],
                             start=False, stop=True)
        return st

    def half_istd(st_ap, dim):
        """bf16 [1,512] row of 1/sqrt(st/dim), broadcast to 128 parts."""
        rec = rowf.tile([1, 512], F32, name="rec", tag="rowf")
        nc.vector.reciprocal_approx_fast(rec[:], st_ap)
        ist = rowi.tile([1, 512], BF, name="ist", tag="rowi")
        nc.scalar.activation(ist[:], rec[:], ACT.Sqrt, scale=float(dim))
        bc = bcp.tile([128, 512], BF, name="bct")
        nc.gpsimd.partition_broadcast(bc[:], ist[0:1, :])
        return bc

    def ln_finish(ev, st, dim, gam, bet, func, out_tile, via_ts=False):
        bc = half_istd(st[:], dim)
        for c, e in enumerate(ev):
            z = zp.tile([128, 512], BF, name="zzt")
            nc.vector.tensor_mul(z[:], e[:], bc[:])
            if via_ts:
                nc.vector.tensor_scalar(out_tile[:, c, :], z[:],
                                        gam[:, c:c + 1], bet[:, c:c + 1],
                                        ALU.mult, ALU.add)
            else:
                nc.scalar.activation(out_tile[:, c, :], z[:], func,
                                     bias=bet[:, c:c + 1],
                                     scale=gam[:, c:c + 1])

    # ---------------- per-stage state ----------------
    yh = [[None] * 2 for _ in range(3)]
    e_ = [[None] * 2 for _ in range(3)]
    m_ = [[None] * 2 for _ in range(3)]
    zt = [[None] * 2 for _ in range(3)]
    l_ = [[None] * 2 for _ in range(3)]
    h_ = [None] * 2
    wrow = [[None] * 3 for _ in range(2)]
    hpw = [None] * 3
    mw = [None] * 3
    gw = [None] * 3
    rw = [None]
    f1w = [None]
    f2w = [None]
    hpx = [[None] * 2 for _ in range(3)]

    def lp_group(s, bt):
        bsl = ts(bt, 512)
        lt = ltp.tile([FS[s], 512], F32, name="ltt", tag="ltt")
        nc.sync.dma_start(lt[:], io[f"lT{s}"][:, bsl])
        lsg = ltp.tile([FS[s], 512], BF, name="lsg", tag="ltt")
        nc.scalar.activation(lsg[:], lt[:], ACT.Sigmoid)
        ps = [mm_ps.tile([128, 512], F32, name="mm") for _ in range(2)]
        for m in range(2):
            nc.tensor.matmul(ps[m][:], w_lp[s][:, ts(m, 128)], lsg[:],
                             start=True, stop=True)
        ev, sq = evict_sq(ps, [b_lp[:, s, c:c + 1] for c in range(2)])

        def fin(s=s, bt=bt, ev=ev, sq=sq):
            st = stats(sq, add_eps=True)
            l_sb = lpool.tile([128, 2, 512], BF, name="l_sb")
            ln_finish(ev, st, D // 2, g_lp[:, s], be_lp[:, s], ACT.Gelu,
                      l_sb)
            l_[s][bt] = l_sb
        pend.append(fin)

    def hp_prefetch(s, bt):
        xc = xpool.tile([128, 4, 2, 512], BF, name="xc")
        nc.sync.dma_start(xc[:], io[f"xT{s}"][bt])
        hpx[s][bt] = xc
        if bt == 0:
            hpw[s] = load_pairs(io["w_hp"][s], 4)

    def hp_group(s, bt):
        xc = hpx[s][bt]
        ps = emit_mms(hpw[s], lambda c: xc[:, c // 2, c % 2, :])
        ev, sq = evict_sq(ps, [b_hp[:, s, c:c + 1] for c in range(4)])

        def fin(s=s, bt=bt, ev=ev, sq=sq):
            st = stats(sq)
            y_sb = big.tile([128, 4, 512], BF, name="big_sb")
            ln_finish(ev, st, D, g_hp[:, s], be_hp[:, s], ACT.Gelu, y_sb)
            yh[s][bt] = y_sb
        pend.append(fin)

    def r_group(s, bt):
        if rw[0] is None:
            rw[0] = load_pairs(io["w_r"], 2)
        ps = emit_mms(rw[0], lambda c: yh[s][bt][:, c, :])
        ev, sq = evict_sq(ps, [b_r[:, c:c + 1] for c in range(4)])

        def fin(s=s, bt=bt, ev=ev, sq=sq):
            st = stats(sq)
            bc = half_istd(st[:], D)
            e_sb = big.tile([128, 4, 512], BF, name="big_sb")
            for c, et in enumerate(ev):
                nc.vector.scalar_tensor_tensor(e_sb[:, c, :], et[:],
                                               g_n1[:, s, c:c + 1], bc[:],
                                               ALU.mult, ALU.mult)
            e_[s][bt] = e_sb
        pend.append(fin)

    m_streams = [(1, 2), (0, 2), (0, 1)]

    def m_group(s, bt):
        if bt == 0:
            mw[s] = load_pairs(io["w_m"][s], 4)
        sa, sb = m_streams[s]
        ps = emit_mms(mw[s], lambda c: (e_[sa][bt][:, c, :] if c < 4
                                        else e_[sb][bt][:, c - 4, :]))
        ev, _ = evict_sq(ps, [b_m[:, s, c:c + 1] for c in range(4)],
                         do_sq=False, pool=mp)
        m_[s][bt] = ev

    def g_group(s, bt):
        if bt == 0:
            gw[s] = load_pairs(io["w_g"][s], 4)
        ps = emit_mms(gw[s], lambda c: (e_[s][bt][:, c, :] if c < 4
                                        else m_[s][bt][c - 4][:]))
        t_sb = tpool.tile([128, 4, 512], BF, name="t_sb")
        sqs = []
        for c in range(4):
            sg = sgp.tile([128, 512], BF, name="sgt")
            nc.scalar.activation(sg[:], ps[c][:], ACT.Sigmoid,
                                 bias=b_g[:, s, c:c + 1])
            q = qp.tile([128, 512], BF, name="qt")
            nc.vector.tensor_mul(q[:], sg[:], m_[s][bt][c][:])
            nc.vector.scalar_tensor_tensor(t_sb[:, c, :],
                                           e_[s][bt][:, c, :],
                                           be_n1[:, s, c:c + 1], q[:],
                                           ALU.add, ALU.add)
            sqc = sqp.tile([128, 512], BF, name="sqt")
            nc.vector.tensor_mul(sqc[:], t_sb[:, c, :], t_sb[:, c, :])
            sqs.append(sqc)

        def fin(s=s, bt=bt, t_sb=t_sb, sqs=sqs):
            st_sum = st_ps.tile([1, 512], F32, name="st", tag="stps")
            for c in range(4):
                nc.tensor.matmul(st_sum[:], ones_col[:], t_sb[:, c, :],
                                 start=(c == 0), stop=(c == 3))
            st_sq = stats(sqs)
            mu = rowf.tile([1, 512], F32, name="mu", tag="rowf")
            nc.scalar.activation(mu[:], st_sum[:], ACT.Copy, scale=1.0 / D)
            v = rowf.tile([1, 512], F32, name="vv", tag="rowf")
            m2 = rowf.tile([1, 512], F32, name="m2", tag="rowf")
            nc.vector.tensor_mul(m2[:], mu[:], mu[:])
            nc.vector.scalar_tensor_tensor(v[:], m2[:], -float(D), st_sq[:],
                                           ALU.mult, ALU.add)
            rec = rowf.tile([1, 512], F32, name="rec", tag="rowf")
            nc.vector.reciprocal_approx_fast(rec[:], v[:])
            ist = rowi.tile([1, 512], BF, name="ist", tag="rowi")
            nc.scalar.activation(ist[:], rec[:], ACT.Sqrt, scale=float(D))
            wr = wrp.tile([1, 512], BF, name="wr1")
            nc.vector.tensor_mul(wr[:], mu[:], ist[:])
            wrow[bt][s] = wr
            bc = bcp.tile([128, 512], BF, name="bct")
            nc.gpsimd.partition_broadcast(bc[:], ist[0:1, :])
            zt_sb = big.tile([128, 4, 512], BF, name="big_sb")
            for c in range(4):
                nc.vector.tensor_mul(zt_sb[:, c, :], t_sb[:, c, :], bc[:])
            zt[s][bt] = zt_sb
        pend.append(fin)

    def f1_group(bt):
        if bt == 0:
            wc = f1p.tile([128, 9, 2, 512], BF, name="f1wc")
            nc.sync.dma_start(wc[:], io["w_f1all"])
            mk = lambda pi, cc: (lambda m, pi=pi, cc=cc:
                                 wc[:, pi, cc, ts(m, 128)])
            f1lw = [[mk(s, c) for c in range(2)] for s in range(3)]
            f1ww = [[mk(3 + 2 * s + c // 2, c % 2) for c in range(4)]
                    for s in range(3)]
            f1w[0] = (f1lw, f1ww)
        f1lw, f1ww = f1w[0]
        ps = [mm_ps.tile([128, 512], F32, name="mm") for _ in range(4)]
        seq = []
        for s in range(3):
            seq += [(f1lw[s][c], l_[s][bt][:, c, :]) for c in range(2)]
        for s in (2, 1, 0):
            seq += [(f1ww[s][c], zt[s][bt][:, c, :]) for c in range(4)]
        for s in (2, 1, 0):
            seq.append((lambda m, s=s: negc3[0:1, s, ts(m, 128)],
                        wrow[bt][s][:]))
        last = len(seq) - 1
        for ci, (lf, rhs) in enumerate(seq):
            for m in range(4):
                nc.tensor.matmul(ps[m][:], lf(m), rhs,
                                 start=(ci == 0), stop=(ci == last))
        ev, sq = evict_sq(ps, [b_f1[:, c:c + 1] for c in range(4)])

        def fin(bt=bt, ev=ev, sq=sq):
            st = stats(sq)
            h_sb = big.tile([128, 4, 512], BF, name="big_sb")
            ln_finish(ev, st, D, g_f1, be_f1, ACT.Gelu, h_sb)
            h_[bt] = h_sb
        pend.append(fin)

    def f2_group(bt):
        bsl = ts(bt, 512)
        if bt == 0:
            f2w[0] = load_pairs(io["w_f2"], 2)
        ps = emit_mms(f2w[0], lambda c: h_[bt][:, c, :])
        ev, sq = evict_sq(ps, [b_f2[:, c:c + 1] for c in range(4)])

        def fin(bt=bt, bsl=bsl, ev=ev, sq=sq):
            st = stats(sq)
            bc = half_istd(st[:], D)
            for c, et in enumerate(ev):
                z = op_.tile([128, 512], F32, name="ot")
                nc.vector.tensor_mul(z[:], et[:], bc[:])
                o = op_.tile([128, 512], F32, name="ot")
                nc.vector.tensor_scalar(o[:], z[:], g_f2[:, c:c + 1],
                                        be_f2[:, c:c + 1], ALU.mult, ALU.add)
                nc.sync.dma_start(io["outT"][ts(c, 128), bsl], o[:])
        pend.append(fin)

    # ---------------- emission schedule ----------------
    # One-behind-ish flushing: fin(G) is flushed ~2 steps after G, always
    # >=2 steps before G's consumer. PSUM banks are freed by the inline
    # evictions, so delayed fins never gate bank reuse.
    hp_prefetch(0, 0); hp_prefetch(0, 1)
    hp_group(0, 0)
    hp_prefetch(1, 0)
    hp_group(0, 1)
    hp_prefetch(1, 1)
    hp_group(1, 0)
    hp_group(1, 1); flush(1)      # hp00
    hp_prefetch(2, 0); hp_prefetch(2, 1)
    hp_group(2, 0); flush(1)      # hp01
    hp_group(2, 1); flush(1)      # hp10
    r_group(0, 0); flush(1)       # hp11
    lp_group(0, 0)
    r_group(1, 0); flush(1)       # hp20
    lp_group(0, 1)
    r_group(2, 0); flush(1)       # hp21
    lp_group(1, 0)
    r_group(0, 1); flush(2)       # r00 -> e0b0, lp00
    lp_group(1, 1)
    r_group(1, 1); flush(2)       # r10 -> e1b0, lp01
    lp_group(2, 0)
    m_group(2, 0); flush(2)       # r20 -> e2b0, lp10
    lp_group(2, 1)
    r_group(2, 1); flush(2)       # r01 -> e0b1, lp11
    flush(2)                      # r11 -> e1b1, lp20
    m_group(2, 1)
    flush(2)                      # r21 -> e2b1, lp21
    g_group(2, 0)
    g_group(2, 1)
    m_group(1, 0)
    m_group(1, 1); flush(1)       # g20 -> zt2b0
    g_group(1, 0); flush(1)       # g21 -> zt2b1
    g_group(1, 1)
    m_group(0, 0)
    m_group(0, 1); flush(1)       # g10 -> zt1b0
    g_group(0, 0); flush(1)       # g11 -> zt1b1
    g_group(0, 1); flush(1)       # g00 -> zt0b0
    f1_group(0); flush(2)         # g01 -> zt0b1, f1(0) -> h0
    f1_group(1); flush(1)         # f1(1) -> h1
    f2_group(0); flush(1)         # f2(0) -> out b0
    f2_group(1)
    flush()
    ctx.close()


def build_program():
    nc = bacc.Bacc("TRN2", target_bir_lowering=False, debug=False,
                   num_devices=NCORES)
    io = {}

    def din(name, shape, dtype=F32):
        io[name] = nc.dram_tensor(name, list(shape), dtype,
                                  kind="ExternalInput").ap()

    for s in range(3):
        din(f"xT{s}", (2, 128, 4, 2, 512), dtype=BF)
        din(f"lT{s}", (FS[s], BL))
    din("w_hp", (3, 128, 4, 2, 512), dtype=BF)
    din("w_r", (128, 2, 2, 512), dtype=BF)
    din("w_m", (3, 128, 4, 2, 512), dtype=BF)
    din("w_g", (3, 128, 4, 2, 512), dtype=BF)
    for s in range(3):
        din(f"w_lp{s}", (FS[s], 256), dtype=BF)
    din("w_f1all", (128, 9, 2, 512), dtype=BF)
    din("negc_f1", (1, 3, 512), dtype=BF)
    din("w_f2", (128, 2, 2, 512), dtype=BF)
    din("cvec", (128, CVEC_NCOLS))
    din("ones_col", (128, 1), dtype=BF)
    din("eps_lhs", (1, 1), dtype=BF)
    din("one_row", (1, 512), dtype=BF)
    io["outT"] = nc.dram_tensor("outT", [D, BL], F32,
                                kind="ExternalOutput").ap()

    with tile.TileContext(nc) as tc:
        emit_program(tc, io)
    nc.compile()
    return nc


def make_in_maps(inputs):
    fw = fold_weights(inputs)
    dev = device_arrays(fw)
    hidden = [np.asarray(inputs["verb_hidden"], np.float32),
              np.asarray(inputs["inst_hidden"], np.float32),
              np.asarray(inputs["target_hidden"], np.float32)]
    logits = [np.asarray(inputs["verb_logits"], np.float32),
              np.asarray(inputs["inst_logits"], np.float32),
              np.asarray(inputs["target_logits"], np.float32)]
    in_maps = []
    for core in range(NCORES):
        rows = slice(core * BL, (core + 1) * BL)
        m = dict(dev)
        for s in range(3):
            xm = hidden[s][rows].T.reshape(4, 2, 128, 2, 512)
            m[f"xT{s}"] = np.ascontiguousarray(
                xm.transpose(3, 2, 0, 1, 4)).astype(BF_NP)
            m[f"lT{s}"] = np.ascontiguousarray(logits[s][rows].T)
        in_maps.append(m)
    return in_maps


_NC_CACHE = None


def _run(inputs, **spmd_kwargs):
    global _NC_CACHE
    if _NC_CACHE is None:
        _NC_CACHE = build_program()
    nc = _NC_CACHE
    in_maps = make_in_maps(inputs)
    res = run_bass_kernel_spmd(nc, in_maps, list(range(NCORES)),
                               **spmd_kwargs)
    out = np.empty((B, D), dtype=np.float32)
    for core in range(NCORES):
        out[core * BL:(core + 1) * BL] = res.results[core]["outT"].T
    return out, res


def kernel(**inputs) -> np.ndarray:
    return _run(inputs)[0]


def kernel_profiled(inputs, tmpdir=None):
    """Returns (out, BassKernelResults) with an NTFF-based profile."""
    return _run(inputs, trace=True, tmpdir=tmpdir)


# revision 41
# speedup vs baseline: 1.3358x; 1.0151x over previous
"""Trainium2 Bass kernel for nn_AttentionModule_7146825580577.

Strategy: pure data parallel over the batch dim (8192 rows -> 1024 rows
per core, 8 cores), weights replicated.

Device math (per core), feature-transposed layout (features on SBUF
partitions, batch on the free dim), bf16 matmul operands with fp32 PSUM
accumulation:

  - LayerNorms over affine-of-activation inputs use host-side
    column-centered weights, so mean(y) == 0 by construction and only
    sum(y^2) is needed (ones-vector matmul on the PE).
  - seq_len==1 MHA reduces to out_proj(v_proj(kv)); fused on the host
    into single 512x512 matrices; self-attention residual folded as
    I + Wv@Wo.
  - The n2 LayerNorm (post-gating) is folded into the fus_W1 matmul:
    gamma scales fold into weights, the per-sample mean correction is a
    rank-1 matmul term (k=3 packed), betas fold into the bias.
  - 1/sqrt(var) via DVE reciprocal_approx_fast + ACT Sqrt on [1,512]
    stat rows (no PE transposes, no NR loop); istd broadcast across
    partitions on GPSIMD.
  - The two 512-column batch tiles are processed in lockstep per
    (stage, stream) group so each weight chunk is DMA'd from HBM once
    and consumed by both tiles back-to-back; LN chains of group k
    execute under the matmuls of group k+1, keeping the PE dense (and
    the HAM clock-gate warm).
"""
import os
import sys

sys.path.insert(0, "/opt/trn_rl_repo")

import numpy as np
import ml_dtypes

import concourse.bass as bass
import concourse.tile as tile
from concourse import bacc, mybir
from concourse.bass import ts
from concourse.bass_utils import run_bass_kernel_spmd

D = 512
HID = 1024
B = 8192
NCORES = 8
BL = B // NCORES          # rows per core
NBT = BL // D             # batch tiles per core (2)
EPS = 1e-5
F32 = mybir.dt.float32
BF = mybir.dt.bfloat16
FS = [10, 6, 15]          # logit dims per stream
F64 = np.float64
BF_NP = ml_dtypes.bfloat16


# --------------------------------------------------------------------------
# Host-side weight folding (float64)
# --------------------------------------------------------------------------

def _center_cols(W, b):
    W = np.asarray(W, F64)
    b = np.asarray(b, F64)
    return W - W.mean(axis=1, keepdims=True), b - b.mean()


def fold_weights(inp):
    g = lambda k: np.asarray(inp[k], dtype=F64)
    out = {}

    w_hp, b_hp = [], []
    for s in range(3):
        W, b = _center_cols(g("hp_W")[s], g("hp_b")[s])
        w_hp.append(W)
        b_hp.append(b)
    out["w_hp"] = np.stack(w_hp)
    out["b_hp"] = np.stack(b_hp)
    out["g_hp"], out["be_hp"] = g("hp_g"), g("hp_be")

    mhaW, mhab = g("mha_in_W"), g("mha_in_b")
    moW, mob = g("mha_out_W"), g("mha_out_b")
    Wv0, bv0 = mhaW[0][:, 2 * D:], mhab[0][2 * D:]
    Wr, br = _center_cols(np.eye(D) + Wv0 @ moW[0], bv0 @ moW[0] + mob[0])
    out["w_r"], out["b_r"] = Wr, br
    out["g_n1"], out["be_n1"] = g("n1_g"), g("n1_be")

    Wj, bj = [None] * 4, [None] * 4
    for j in (1, 2, 3):
        Wv, bv = mhaW[j][:, 2 * D:], mhab[j][2 * D:]
        Wj[j] = Wv @ moW[j]
        bj[j] = bv @ moW[j] + mob[j]
    # m_verb uses (inst_e, target_e); m_inst (verb, target); m_target (verb, inst)
    mods = [(1, 2), (1, 3), (2, 3)]
    streams = [(1, 2), (0, 2), (0, 1)]
    be1 = g("n1_be")
    w_m, b_m = [], []
    for s in range(3):
        ja, jb = mods[s]
        sa, sb = streams[s]
        w_m.append(np.concatenate([0.5 * Wj[ja], 0.5 * Wj[jb]], axis=0))
        # device e-tiles carry only g*z (be_n1 folded here)
        b_m.append(0.5 * (bj[ja] + bj[jb])
                   + 0.5 * (be1[sa] @ Wj[ja] + be1[sb] @ Wj[jb]))
    out["w_m"] = np.stack(w_m)
    out["b_m"] = np.stack(b_m)

    gW = g("gate_W")
    out["w_g"] = gW
    out["b_g"] = g("gate_b") + np.stack(
        [be1[s] @ gW[s][:D] for s in range(3)])

    w_lp, b_lp = [], []
    for s, key in enumerate(["verb", "inst", "target"]):
        W, b = _center_cols(g(f"lp_W_{key}"), g(f"lp_b_{key}"))
        w_lp.append(W)
        b_lp.append(b)
    out["w_lp"] = w_lp
    out["b_lp"] = np.stack(b_lp)
    out["g_lp"], out["be_lp"] = g("lp_g"), g("lp_be")

    W1 = g("fus_W1")
    g2, be2 = g("n2_g"), g("n2_be")
    A1, negc = [], []
    bias_total = g("fus_b1").copy()
    for s in range(3):
        blk = W1[s * D:(s + 1) * D]
        A = g2[s][:, None] * blk
        c = blk.T @ g2[s]
        A1.append(A - A.mean(axis=1, keepdims=True))
        negc.append(-(c - c.mean()))
        bias_total += be2[s] @ blk
    L1 = []
    for s in range(3):
        off = 3 * D + s * (D // 2)
        blk = W1[off: off + D // 2]
        L1.append(blk - blk.mean(axis=1, keepdims=True))
    out["w_f1"] = np.stack(A1)
    out["negc_f1"] = np.stack(negc)
    out["w_f1l"] = np.stack(L1)
    out["b_f1"] = bias_total - bias_total.mean()
    out["g_f1"], out["be_f1"] = g("fus_g1"), g("fus_ge1")

    W2c, b2c = _center_cols(g("fus_W2"), g("fus_b2"))
    out["w_f2"], out["b_f2"] = W2c, b2c
    out["g_f2"], out["be_f2"] = g("fus_g2"), g("fus_ge2")
    return out


def _vec_pp(v, nk):
    """[.., nk*128] feature vector -> per-partition layout [.., 128, nk]."""
    v = np.asarray(v, np.float32)
    return np.ascontiguousarray(v.reshape(v.shape[:-1] + (nk, 128)).swapaxes(-1, -2))


CVEC_SPEC = [("b_hp", 4, 3), ("b_m", 4, 3), ("g_hp", 4, 3),
             ("be_hp", 4, 3), ("g_n1", 4, 3), ("be_n1", 4, 3),
             ("b_g", 4, 3), ("b_r", 4, 1), ("b_f1", 4, 1),
             ("b_f2", 4, 1), ("g_f1", 4, 1), ("be_f1", 4, 1),
             ("g_f2", 4, 1), ("be_f2", 4, 1), ("b_lp", 2, 3),
             ("g_lp", 2, 3), ("be_lp", 2, 3)]
CVEC_OFF = {}
_off = 0
for _n, _k, _s in CVEC_SPEC:
    CVEC_OFF[_n] = (_off, _k * _s)
    _off += _k * _s
CVEC_NCOLS = _off


def device_arrays(fw):
    f32 = lambda v: np.ascontiguousarray(np.asarray(v, np.float32))
    bf = lambda v: np.ascontiguousarray(
        np.asarray(v, np.float32).astype(BF_NP))
    def pairs(W, nk):
        # [nk*128, 512] -> [128, nk/2, 2, 512]: whole stage in ONE DMA,
        # contiguous nk*1KB run per partition.
        W = np.asarray(W, np.float32)
        return bf(W.reshape(nk // 2, 2, 128, 512).transpose(2, 0, 1, 3))
    dev = {}
    dev["w_hp"] = bf(np.stack([pairs(fw["w_hp"][s], 8) for s in range(3)]))
    dev["w_r"] = pairs(fw["w_r"], 4)
    dev["w_m"] = bf(np.stack([pairs(fw["w_m"][s], 8) for s in range(3)]))
    dev["w_g"] = bf(np.stack([pairs(fw["w_g"][s], 8) for s in range(3)]))
    for s in range(3):
        dev[f"w_lp{s}"] = bf(fw["w_lp"][s])
    dev["w_f1all"] = bf(np.concatenate(
        [pairs(fw["w_f1l"][s], 2) for s in range(3)]
        + [pairs(fw["w_f1"][s], 4) for s in range(3)], axis=1))
    dev["negc_f1"] = bf(fw["negc_f1"][None])     # [1, 3, 512]
    dev["w_f2"] = pairs(fw["w_f2"], 4)
    # all per-partition bias/gamma vectors in one [128, ncols] tensor
    cols = []
    for name, nk, _ns in CVEC_SPEC:
        v = _vec_pp(fw[name], nk)
        v = v.reshape(128, -1) if v.ndim == 2 else \
            np.ascontiguousarray(v.transpose(1, 0, 2)).reshape(128, -1)
        assert v.shape[1] == CVEC_OFF[name][1], name
        cols.append(v)
    dev["cvec"] = np.ascontiguousarray(np.concatenate(cols, axis=1))
    dev["ones_col"] = np.ones((128, 1), BF_NP)
    dev["eps_lhs"] = np.full((1, 1), (D // 2) * EPS, BF_NP)
    dev["one_row"] = np.ones((1, 512), BF_NP)
    return dev


# --------------------------------------------------------------------------
# Device program
# --------------------------------------------------------------------------

def emit_program(tc, io):
    nc = tc.nc
    ACT = mybir.ActivationFunctionType
    ALU = mybir.AluOpType
    from contextlib import ExitStack
    ctx = ExitStack()

    P = lambda name, bufs, space="SBUF": ctx.enter_context(
        tc.tile_pool(name=name, bufs=bufs, space=space))
    const = P("const", 1)
    wpool = P("w", 3)
    xpool = P("x", 2)
    lpool = P("l", 6)
    big = P("big", 12)
    tpool = P("t", 2)
    mp = P("m", 8)
    evp = P("ev", 8)
    sqp = P("sq", 5)
    zp = P("z", 6)
    qp = P("q", 4)
    sgp = P("sg", 4)
    bcp = P("bc", 4)
    op_ = P("o", 4)
    rowf = P("rowf", 3)
    rowi = P("rowi", 3)
    wrp = P("wr", 6)
    ltp = P("lt", 4)
    f1p = P("f1w", 1)
    mm_ps = P("mm_ps", 6, "PSUM")
    st_ps = P("st_ps", 2, "PSUM")

    # ---------------- earliest DMAs: first stage inputs/weights ----------
    # (issued before the const loads so the Sync engine's serial trigger
    # stream starts the big startup transfers first)
    x00 = xpool.tile([128, 4, 2, 512], BF, name="xc")
    nc.sync.dma_start(x00[:], io["xT0"][0])
    x01 = xpool.tile([128, 4, 2, 512], BF, name="xc")
    nc.sync.dma_start(x01[:], io["xT0"][1])
    w0 = wpool.tile([128, 4, 2, 512], BF, name="wc", tag="wc")
    nc.sync.dma_start(w0[:], io["w_hp"][0])

    # ---------------- constants ----------------
    def load(name, shape, rearr=None, dtype=F32):
        t = const.tile(shape, dtype, name=name)
        src = io[name]
        if rearr:
            src = src.rearrange(rearr)
        nc.sync.dma_start(t[:], src)
        return t

    ones_col = load("ones_col", [128, 1], dtype=BF)
    eps_lhs = load("eps_lhs", [1, 1], dtype=BF)
    one_row = load("one_row", [1, 512], dtype=BF)
    negc3 = load("negc_f1", [1, 3, 512], dtype=BF)
    cvec = load("cvec", [128, CVEC_NCOLS])

    def cv(name):
        off, n = CVEC_OFF[name]
        ap = cvec[:, off:off + n]
        if n > 4:
            ap = ap.rearrange("p (s c) -> p s c", s=3)
        return ap

    b_hp, b_m, g_hp, be_hp = cv("b_hp"), cv("b_m"), cv("g_hp"), cv("be_hp")
    g_n1, be_n1, b_g = cv("g_n1"), cv("be_n1"), cv("b_g")
    b_r, b_f1, b_f2 = cv("b_r"), cv("b_f1"), cv("b_f2")
    g_f1, be_f1, g_f2, be_f2 = cv("g_f1"), cv("be_f1"), cv("g_f2"), cv("be_f2")
    b_lp, g_lp, be_lp = cv("b_lp"), cv("g_lp"), cv("be_lp")
    w_lp = [load(f"w_lp{s}", [FS[s], 256], dtype=BF) for s in range(3)]

    # ---------------- helpers ----------------
    pend = []

    def flush(n=None):
        cnt = len(pend) if n is None else n
        for _ in range(cnt):
            if pend:
                pend.pop(0)()

    def load_pairs(dram_stage, npairs):
        """One DMA for a whole [128, npairs, 2, 512] weight stage."""
        wc = wpool.tile([128, npairs, 2, 512], BF, name="wc", tag="wc")
        nc.sync.dma_start(wc[:], dram_stage)
        fns = []
        for i in range(npairs):
            for cc in range(2):
                fns.append(lambda m, wc=wc, i=i, cc=cc:
                           wc[:, i, cc, ts(m, 128)])
        return fns

    def emit_mms(lhs_fns, rhs_fn, nm=4):
        ps = [mm_ps.tile([128, 512], F32, name="mm") for _ in range(nm)]
        last = len(lhs_fns) - 1
        for ci, lf in enumerate(lhs_fns):
            rhs = rhs_fn(ci)
            for m in range(nm):
                nc.tensor.matmul(ps[m][:], lf(m), rhs,
                                 start=(ci == 0), stop=(ci == last))
        return ps

    def evict_sq(ps_list, bias_cols, do_sq=True, pool=None, dve_half=True,
                 sq_ps=False):
        """Evict psum chunks (+bias) to bf16 SBUF; optionally square them.
        Evictions alternate ACT/DVE so the last chunk lands fast. With
        sq_ps, squares come straight from PSUM on ACT (Square is in every
        activation table, so no table reload)."""
        ev, sq = [], []
        for c, psx in enumerate(ps_list):
            e = (pool or evp).tile([128, 512], BF, name="evt")
            if dve_half and c % 2 == 1:
                nc.vector.tensor_scalar_add(e[:], psx[:], bias_cols[c])
            else:
                nc.scalar.activation(e[:], psx[:], ACT.Identity,
                                     bias=bias_cols[c])
            ev.append(e)
            if do_sq:
                s = sqp.tile([128, 512], BF, name="sqt")
                if sq_ps:
                    nc.scalar.activation(s[:], psx[:], ACT.Square,
                                         bias=bias_cols[c])
                else:
                    nc.vector.tensor_mul(s[:], e[:], e[:])
                sq.append(s)
        return ev, sq

    def stats(sq_list, add_eps=False):
        st = st_ps.tile([1, 512], F32, name="st", tag="stps")
        n = len(sq_list) + (1 if add_eps else 0)
        for c, s in enumerate(sq_list):
            nc.tensor.matmul(st[:], ones_col[:], s[:],
                             start=(c == 0), stop=(c == n - 1))
        if add_eps:
            nc.tensor.matmul(st[:], eps_lhs[:], one_row[:],
                             start=False, stop=True)
        return st

    def half_istd(st_ap, dim):
        """bf16 [1,512] row of 1/sqrt(st/dim), broadcast to 128 parts."""
        rec = rowf.tile([1, 512], F32, name="rec", tag="rowf")
        nc.vector.reciprocal_approx_fast(rec[:], st_ap)
        ist = rowi.tile([1, 512], BF, name="ist", tag="rowi")
        nc.scalar.activation(ist[:], rec[:], ACT.Sqrt, scale=float(dim))
        bc = bcp.tile([128, 512], BF, name="bct")
        nc.gpsimd.partition_broadcast(bc[:], ist[0:1, :])
        return bc

    def ln_finish(ev, st, dim, gam, bet, func, out_tile, via_ts=False):
        bc = half_istd(st[:], dim)
        for c, e in enumerate(ev):
            z = zp.tile([128, 512], BF, name="zzt")
            nc.vector.tensor_mul(z[:], e[:], bc[:])
            if via_ts:
                nc.vector.tensor_scalar(out_tile[:, c, :], z[:],
                                        gam[:, c:c + 1], bet[:, c:c + 1],
                                        ALU.mult, ALU.add)
            else:
                nc.scalar.activation(out_tile[:, c, :], z[:], func,
                                     bias=bet[:, c:c + 1],
                                     scale=gam[:, c:c + 1])

    # ---------------- per-stage state ----------------
    yh = [[None] * 2 for _ in range(3)]
    e_ = [[None] * 2 for _ in range(3)]
    m_ = [[None] * 2 for _ in range(3)]
    zt = [[None] * 2 for _ in range(3)]
    l_ = [[None] * 2 for _ in range(3)]
    h_ = [None] * 2
    wrow = [[None] * 3 for _ in range(2)]
    hpw = [None] * 3
    mw = [None] * 3
    gw = [None] * 3
    rw = [None]
    f1w = [None]
    f2w = [None]
    hpx = [[None] * 2 for _ in range(3)]

    def lp_group(s, bt):
        bsl = ts(bt, 512)
        lt = ltp.tile([FS[s], 512], F32, name="ltt", tag="ltt")
        nc.sync.dma_start(lt[:], io[f"lT{s}"][:, bsl])
        lsg = ltp.tile([FS[s], 512], BF, name="lsg", tag="ltt")
        nc.scalar.activation(lsg[:], lt[:], ACT.Sigmoid)
        ps = [mm_ps.tile([128, 512], F32, name="mm") for _ in range(2)]
        for m in range(2):
            nc.tensor.matmul(ps[m][:], w_lp[s][:, ts(m, 128)], lsg[:],
                             start=True, stop=True)
        ev, sq = evict_sq(ps, [b_lp[:, s, c:c + 1] for c in range(2)])

        def fin(s=s, bt=bt, ev=ev, sq=sq):
            st = stats(sq, add_eps=True)
            l_sb = lpool.tile([128, 2, 512], BF, name="l_sb")
            ln_finish(ev, st, D // 2, g_lp[:, s], be_lp[:, s], ACT.Gelu,
                      l_sb)
            l_[s][bt] = l_sb
        pend.append(fin)

    def hp_prefetch(s, bt):
        if s == 0:
            hpx[0] = [x00, x01]
            if bt == 0:
                fns = []
                for i in range(4):
                    for cc in range(2):
                        fns.append(lambda m, i=i, cc=cc:
                                   w0[:, i, cc, ts(m, 128)])
                hpw[0] = fns
            return
        xc = xpool.tile([128, 4, 2, 512], BF, name="xc")
        nc.sync.dma_start(xc[:], io[f"xT{s}"][bt])
        hpx[s][bt] = xc
        if bt == 0:
            hpw[s] = load_pairs(io["w_hp"][s], 4)

    def hp_group(s, bt):
        xc = hpx[s][bt]
        ps = emit_mms(hpw[s], lambda c: xc[:, c // 2, c % 2, :])
        ev, sq = evict_sq(ps, [b_hp[:, s, c:c + 1] for c in range(4)])

        def fin(s=s, bt=bt, ev=ev, sq=sq):
            st = stats(sq)
            y_sb = big.tile([128, 4, 512], BF, name="big_sb")
            ln_finish(ev, st, D, g_hp[:, s], be_hp[:, s], ACT.Gelu, y_sb)
            yh[s][bt] = y_sb
        pend.append(fin)

    def r_group(s, bt):
        if rw[0] is None:
            rw[0] = load_pairs(io["w_r"], 2)
        ps = emit_mms(rw[0], lambda c: yh[s][bt][:, c, :])
        ev, sq = evict_sq(ps, [b_r[:, c:c + 1] for c in range(4)])

        def fin(s=s, bt=bt, ev=ev, sq=sq):
            st = stats(sq)
            bc = half_istd(st[:], D)
            e_sb = big.tile([128, 4, 512], BF, name="big_sb")
            for c, et in enumerate(ev):
                nc.vector.scalar_tensor_tensor(e_sb[:, c, :], et[:],
                                               g_n1[:, s, c:c + 1], bc[:],
                                               ALU.mult, ALU.mult)
            e_[s][bt] = e_sb
        pend.append(fin)

    m_streams = [(1, 2), (0, 2), (0, 1)]

    def m_group(s, bt):
        if bt == 0:
            mw[s] = load_pairs(io["w_m"][s], 4)
        sa, sb = m_streams[s]
        ps = emit_mms(mw[s], lambda c: (e_[sa][bt][:, c, :] if c < 4
                                        else e_[sb][bt][:, c - 4, :]))
        ev, _ = evict_sq(ps, [b_m[:, s, c:c + 1] for c in range(4)],
                         do_sq=False, pool=mp)
        m_[s][bt] = ev

    def g_group(s, bt):
        if bt == 0:
            gw[s] = load_pairs(io["w_g"][s], 4)
        ps = emit_mms(gw[s], lambda c: (e_[s][bt][:, c, :] if c < 4
                                        else m_[s][bt][c - 4][:]))
        t_sb = tpool.tile([128, 4, 512], BF, name="t_sb")
        sqs = []
        for c in range(4):
            sg = sgp.tile([128, 512], BF, name="sgt")
            nc.scalar.activation(sg[:], ps[c][:], ACT.Sigmoid,
                                 bias=b_g[:, s, c:c + 1])
            q = qp.tile([128, 512], BF, name="qt")
            nc.vector.tensor_mul(q[:], sg[:], m_[s][bt][c][:])
            nc.vector.scalar_tensor_tensor(t_sb[:, c, :],
                                           e_[s][bt][:, c, :],
                                           be_n1[:, s, c:c + 1], q[:],
                                           ALU.add, ALU.add)
            sqc = sqp.tile([128, 512], BF, name="sqt")
            nc.vector.tensor_mul(sqc[:], t_sb[:, c, :], t_sb[:, c, :])
            sqs.append(sqc)

        def fin(s=s, bt=bt, t_sb=t_sb, sqs=sqs):
            st_sum = st_ps.tile([1, 512], F32, name="st", tag="stps")
            for c in range(4):
                nc.tensor.matmul(st_sum[:], ones_col[:], t_sb[:, c, :],
                                 start=(c == 0), stop=(c == 3))
            st_sq = stats(sqs)
            mu = rowf.tile([1, 512], F32, name="mu", tag="rowf")
            nc.scalar.activation(mu[:], st_sum[:], ACT.Copy, scale=1.0 / D)
            v = rowf.tile([1, 512], F32, name="vv", tag="rowf")
            m2 = rowf.tile([1, 512], F32, name="m2", tag="rowf")
            nc.vector.tensor_mul(m2[:], mu[:], mu[:])
            nc.vector.scalar_tensor_tensor(v[:], m2[:], -float(D), st_sq[:],
                                           ALU.mult, ALU.add)
            rec = rowf.tile([1, 512], F32, name="rec", tag="rowf")
            nc.vector.reciprocal_approx_fast(rec[:], v[:])
            ist = rowi.tile([1, 512], BF, name="ist", tag="rowi")
            nc.scalar.activation(ist[:], rec[:], ACT.Sqrt, scale=float(D))
            wr = wrp.tile([1, 512], BF, name="wr1")
            nc.vector.tensor_mul(wr[:], mu[:], ist[:])
            wrow[bt][s] = wr
            bc = bcp.tile([128, 512], BF, name="bct")
            nc.gpsimd.partition_broadcast(bc[:], ist[0:1, :])
            zt_sb = big.tile([128, 4, 512], BF, name="big_sb")
            for c in range(4):
                nc.vector.tensor_mul(zt_sb[:, c, :], t_sb[:, c, :], bc[:])
            zt[s][bt] = zt_sb
        pend.append(fin)

    def f1_group(bt):
        if bt == 0:
            wc = f1p.tile([128, 9, 2, 512], BF, name="f1wc")
            nc.sync.dma_start(wc[:], io["w_f1all"])
            mk = lambda pi, cc: (lambda m, pi=pi, cc=cc:
                                 wc[:, pi, cc, ts(m, 128)])
            f1lw = [[mk(s, c) for c in range(2)] for s in range(3)]
            f1ww = [[mk(3 + 2 * s + c // 2, c % 2) for c in range(4)]
                    for s in range(3)]
            f1w[0] = (f1lw, f1ww)
        f1lw, f1ww = f1w[0]
        ps = [mm_ps.tile([128, 512], F32, name="mm") for _ in range(4)]
        seq = []
        for s in range(3):
            seq += [(f1lw[s][c], l_[s][bt][:, c, :]) for c in range(2)]
        for s in (2, 1, 0):
            seq += [(f1ww[s][c], zt[s][bt][:, c, :]) for c in range(4)]
        for s in (2, 1, 0):
            seq.append((lambda m, s=s: negc3[0:1, s, ts(m, 128)],
                        wrow[bt][s][:]))
        last = len(seq) - 1
        for ci, (lf, rhs) in enumerate(seq):
            for m in range(4):
                nc.tensor.matmul(ps[m][:], lf(m), rhs,
                                 start=(ci == 0), stop=(ci == last))
        ev, sq = evict_sq(ps, [b_f1[:, c:c + 1] for c in range(4)])

        def fin(bt=bt, ev=ev, sq=sq):
            st = stats(sq)
            h_sb = big.tile([128, 4, 512], BF, name="big_sb")
            ln_finish(ev, st, D, g_f1, be_f1, ACT.Gelu, h_sb)
            h_[bt] = h_sb
        pend.append(fin)

    def f2_group(bt):
        bsl = ts(bt, 512)
        if bt == 0:
            f2w[0] = load_pairs(io["w_f2"], 2)
        ps = emit_mms(f2w[0], lambda c: h_[bt][:, c, :])
        ev, sq = evict_sq(ps, [b_f2[:, c:c + 1] for c in range(4)])

        def fin(bt=bt, bsl=bsl, ev=ev, sq=sq):
            st = stats(sq)
            bc = half_istd(st[:], D)
            for c, et in enumerate(ev):
                z = op_.tile([128, 512], F32, name="ot")
                nc.vector.tensor_mul(z[:], et[:], bc[:])
                o = op_.tile([128, 512], F32, name="ot")
                nc.vector.tensor_scalar(o[:], z[:], g_f2[:, c:c + 1],
                                        be_f2[:, c:c + 1], ALU.mult, ALU.add)
                nc.sync.dma_start(io["outT"][ts(c, 128), bsl], o[:])
        pend.append(fin)

    # ---------------- emission schedule ----------------
    # One-behind-ish flushing: fin(G) is flushed ~2 steps after G, always
    # >=2 steps before G's consumer. PSUM banks are freed by the inline
    # evictions, so delayed fins never gate bank reuse.
    hp_prefetch(0, 0); hp_prefetch(0, 1)
    hp_group(0, 0)
    hp_prefetch(1, 0)
    hp_group(0, 1)
    hp_prefetch(1, 1)
    hp_group(1, 0)
    hp_group(1, 1); flush(1)      # hp00
    hp_prefetch(2, 0); hp_prefetch(2, 1)
    hp_group(2, 0); flush(1)      # hp01
    hp_group(2, 1); flush(1)      # hp10
    r_group(0, 0); flush(1)       # hp11
    lp_group(0, 0)
    r_group(1, 0); flush(1)       # hp20
    lp_group(0, 1)
    r_group(2, 0); flush(1)       # hp21
    lp_group(1, 0)
    r_group(0, 1); flush(2)       # r00 -> e0b0, lp00
    lp_group(1, 1)
    r_group(1, 1); flush(2)       # r10 -> e1b0, lp01
    lp_group(2, 0)
    m_group(2, 0); flush(2)       # r20 -> e2b0, lp10
    lp_group(2, 1)
    r_group(2, 1); flush(2)       # r01 -> e0b1, lp11
    flush(2)                      # r11 -> e1b1, lp20
    m_group(2, 1)
    flush(2)                      # r21 -> e2b1, lp21
    g_group(2, 0)
    g_group(2, 1)
    m_group(1, 0)
    m_group(1, 1); flush(1)       # g20 -> zt2b0
    g_group(1, 0); flush(1)       # g21 -> zt2b1
    g_group(1, 1)
    m_group(0, 0)
    m_group(0, 1); flush(1)       # g10 -> zt1b0
    g_group(0, 0); flush(1)       # g11 -> zt1b1
    g_group(0, 1); flush(1)       # g00 -> zt0b0
    f1_group(0); flush(2)         # g01 -> zt0b1, f1(0) -> h0
    f1_group(1); flush(1)         # f1(1) -> h1
    f2_group(0); flush(1)         # f2(0) -> out b0
    f2_group(1)
    flush()
    ctx.close()


def build_program():
    nc = bacc.Bacc("TRN2", target_bir_lowering=False, debug=False,
                   num_devices=NCORES)
    io = {}

    def din(name, shape, dtype=F32):
        io[name] = nc.dram_tensor(name, list(shape), dtype,
                                  kind="ExternalInput").ap()

    for s in range(3):
        din(f"xT{s}", (2, 128, 4, 2, 512), dtype=BF)
        din(f"lT{s}", (FS[s], BL))
    din("w_hp", (3, 128, 4, 2, 512), dtype=BF)
    din("w_r", (128, 2, 2, 512), dtype=BF)
    din("w_m", (3, 128, 4, 2, 512), dtype=BF)
    din("w_g", (3, 128, 4, 2, 512), dtype=BF)
    for s in range(3):
        din(f"w_lp{s}", (FS[s], 256), dtype=BF)
    din("w_f1all", (128, 9, 2, 512), dtype=BF)
    din("negc_f1", (1, 3, 512), dtype=BF)
    din("w_f2", (128, 2, 2, 512), dtype=BF)
    din("cvec", (128, CVEC_NCOLS))
    din("ones_col", (128, 1), dtype=BF)
    din("eps_lhs", (1, 1), dtype=BF)
    din("one_row", (1, 512), dtype=BF)
    io["outT"] = nc.dram_tensor("outT", [D, BL], F32,
                                kind="ExternalOutput").ap()

    with tile.TileContext(nc) as tc:
        emit_program(tc, io)
    nc.compile()
    return nc


def make_in_maps(inputs):
    fw = fold_weights(inputs)
    dev = device_arrays(fw)
    hidden = [np.asarray(inputs["verb_hidden"], np.float32),
              np.asarray(inputs["inst_hidden"], np.float32),
              np.asarray(inputs["target_hidden"], np.float32)]
    logits = [np.asarray(inputs["verb_logits"], np.float32),
              np.asarray(inputs["inst_logits"], np.float32),
              np.asarray(inputs["target_logits"], np.float32)]
    in_maps = []
    for core in range(NCORES):
        rows = slice(core * BL, (core + 1) * BL)
        m = dict(dev)
        for s in range(3):
            xm = hidden[s][rows].T.reshape(4, 2, 128, 2, 512)
            m[f"xT{s}"] = np.ascontiguousarray(
                xm.transpose(3, 2, 0, 1, 4)).astype(BF_NP)
            m[f"lT{s}"] = np.ascontiguousarray(logits[s][rows].T)
        in_maps.append(m)
    return in_maps


_NC_CACHE = None


def _run(inputs, **spmd_kwargs):
    global _NC_CACHE
    if _NC_CACHE is None:
        _NC_CACHE = build_program()
    nc = _NC_CACHE
    in_maps = make_in_maps(inputs)
    res = run_bass_kernel_spmd(nc, in_maps, list(range(NCORES)),
                               **spmd_kwargs)
    out = np.empty((B, D), dtype=np.float32)
    for core in range(NCORES):
        out[core * BL:(core + 1) * BL] = res.results[core]["outT"].T
    return out, res


def kernel(**inputs) -> np.ndarray:
    return _run(inputs)[0]


def kernel_profiled(inputs, tmpdir=None):
    """Returns (out, BassKernelResults) with an NTFF-based profile."""
    return _run(inputs, trace=True, tmpdir=tmpdir)


# revision 42
# speedup vs baseline: 1.3484x; 1.0094x over previous
"""Trainium2 Bass kernel for nn_AttentionModule_7146825580577.

Strategy: pure data parallel over the batch dim (8192 rows -> 1024 rows
per core, 8 cores), weights replicated.

Device math (per core), feature-transposed layout (features on SBUF
partitions, batch on the free dim), bf16 matmul operands with fp32 PSUM
accumulation:

  - LayerNorms over affine-of-activation inputs use host-side
    column-centered weights, so mean(y) == 0 by construction and only
    sum(y^2) is needed (ones-vector matmul on the PE).
  - seq_len==1 MHA reduces to out_proj(v_proj(kv)); fused on the host
    into single 512x512 matrices; self-attention residual folded as
    I + Wv@Wo.
  - The n2 LayerNorm (post-gating) is folded into the fus_W1 matmul:
    gamma scales fold into weights, the per-sample mean correction is a
    rank-1 matmul term (k=3 packed), betas fold into the bias.
  - 1/sqrt(var) via DVE reciprocal_approx_fast + ACT Sqrt on [1,512]
    stat rows (no PE transposes, no NR loop); istd broadcast across
    partitions on GPSIMD.
  - The two 512-column batch tiles are processed in lockstep per
    (stage, stream) group so each weight chunk is DMA'd from HBM once
    and consumed by both tiles back-to-back; LN chains of group k
    execute under the matmuls of group k+1, keeping the PE dense (and
    the HAM clock-gate warm).
"""
import os
import sys

sys.path.insert(0, "/opt/trn_rl_repo")

import numpy as np
import ml_dtypes

import concourse.bass as bass
import concourse.tile as tile
from concourse import bacc, mybir
from concourse.bass import ts
from concourse.bass_utils import run_bass_kernel_spmd

D = 512
HID = 1024
B = 8192
NCORES = 8
BL = B // NCORES          # rows per core
NBT = BL // D             # batch tiles per core (2)
EPS = 1e-5
F32 = mybir.dt.float32
BF = mybir.dt.bfloat16
FS = [10, 6, 15]          # logit dims per stream
F64 = np.float64
BF_NP = ml_dtypes.bfloat16


# --------------------------------------------------------------------------
# Host-side weight folding (float64)
# --------------------------------------------------------------------------

def _center_cols(W, b):
    W = np.asarray(W, F64)
    b = np.asarray(b, F64)
    return W - W.mean(axis=1, keepdims=True), b - b.mean()


def fold_weights(inp):
    g = lambda k: np.asarray(inp[k], dtype=F64)
    out = {}

    w_hp, b_hp = [], []
    for s in range(3):
        W, b = _center_cols(g("hp_W")[s], g("hp_b")[s])
        w_hp.append(W)
        b_hp.append(b)
    out["w_hp"] = np.stack(w_hp)
    out["b_hp"] = np.stack(b_hp)
    out["g_hp"], out["be_hp"] = g("hp_g"), g("hp_be")

    mhaW, mhab = g("mha_in_W"), g("mha_in_b")
    moW, mob = g("mha_out_W"), g("mha_out_b")
    Wv0, bv0 = mhaW[0][:, 2 * D:], mhab[0][2 * D:]
    Wr, br = _center_cols(np.eye(D) + Wv0 @ moW[0], bv0 @ moW[0] + mob[0])
    out["w_r"], out["b_r"] = Wr, br
    out["g_n1"], out["be_n1"] = g("n1_g"), g("n1_be")

    Wj, bj = [None] * 4, [None] * 4
    for j in (1, 2, 3):
        Wv, bv = mhaW[j][:, 2 * D:], mhab[j][2 * D:]
        Wj[j] = Wv @ moW[j]
        bj[j] = bv @ moW[j] + mob[j]
    # m_verb uses (inst_e, target_e); m_inst (verb, target); m_target (verb, inst)
    mods = [(1, 2), (1, 3), (2, 3)]
    streams = [(1, 2), (0, 2), (0, 1)]
    be1 = g("n1_be")
    w_m, b_m = [], []
    for s in range(3):
        ja, jb = mods[s]
        sa, sb = streams[s]
        w_m.append(np.concatenate([0.5 * Wj[ja], 0.5 * Wj[jb]], axis=0))
        # device e-tiles carry only g*z (be_n1 folded here)
        b_m.append(0.5 * (bj[ja] + bj[jb])
                   + 0.5 * (be1[sa] @ Wj[ja] + be1[sb] @ Wj[jb]))
    out["w_m"] = np.stack(w_m)
    out["b_m"] = np.stack(b_m)

    gW = g("gate_W")
    out["w_g"] = gW
    out["b_g"] = g("gate_b") + np.stack(
        [be1[s] @ gW[s][:D] for s in range(3)])

    w_lp, b_lp = [], []
    for s, key in enumerate(["verb", "inst", "target"]):
        W, b = _center_cols(g(f"lp_W_{key}"), g(f"lp_b_{key}"))
        w_lp.append(W)
        b_lp.append(b)
    out["w_lp"] = w_lp
    out["b_lp"] = np.stack(b_lp)
    out["g_lp"], out["be_lp"] = g("lp_g"), g("lp_be")

    W1 = g("fus_W1")
    g2, be2 = g("n2_g"), g("n2_be")
    A1, negc = [], []
    bias_total = g("fus_b1").copy()
    for s in range(3):
        blk = W1[s * D:(s + 1) * D]
        A = g2[s][:, None] * blk
        c = blk.T @ g2[s]
        A1.append(A - A.mean(axis=1, keepdims=True))
        negc.append(-(c - c.mean()))
        bias_total += be2[s] @ blk
    L1 = []
    for s in range(3):
        off = 3 * D + s * (D // 2)
        blk = W1[off: off + D // 2]
        L1.append(blk - blk.mean(axis=1, keepdims=True))
    out["w_f1"] = np.stack(A1)
    out["negc_f1"] = np.stack(negc)
    out["w_f1l"] = np.stack(L1)
    out["b_f1"] = bias_total - bias_total.mean()
    out["g_f1"], out["be_f1"] = g("fus_g1"), g("fus_ge1")

    W2c, b2c = _center_cols(g("fus_W2"), g("fus_b2"))
    out["w_f2"], out["b_f2"] = W2c, b2c
    out["g_f2"], out["be_f2"] = g("fus_g2"), g("fus_ge2")
    return out


def _vec_pp(v, nk):
    """[.., nk*128] feature vector -> per-partition layout [.., 128, nk]."""
    v = np.asarray(v, np.float32)
    return np.ascontiguousarray(v.reshape(v.shape[:-1] + (nk, 128)).swapaxes(-1, -2))


CVEC_SPEC = [("b_hp", 4, 3), ("b_m", 4, 3), ("g_hp", 4, 3),
             ("be_hp", 4, 3), ("g_n1", 4, 3), ("be_n1", 4, 3),
             ("b_g", 4, 3), ("b_r", 4, 1), ("b_f1", 4, 1),
             ("b_f2", 4, 1), ("g_f1", 4, 1), ("be_f1", 4, 1),
             ("g_f2", 4, 1), ("be_f2", 4, 1), ("b_lp", 2, 3),
             ("g_lp", 2, 3), ("be_lp", 2, 3)]
CVEC_OFF = {}
_off = 0
for _n, _k, _s in CVEC_SPEC:
    CVEC_OFF[_n] = (_off, _k * _s)
    _off += _k * _s
CVEC_NCOLS = _off


def device_arrays(fw):
    f32 = lambda v: np.ascontiguousarray(np.asarray(v, np.float32))
    bf = lambda v: np.ascontiguousarray(
        np.asarray(v, np.float32).astype(BF_NP))
    def pairs(W, nk):
        # [nk*128, 512] -> [128, nk/2, 2, 512]: whole stage in ONE DMA,
        # contiguous nk*1KB run per partition.
        W = np.asarray(W, np.float32)
        return bf(W.reshape(nk // 2, 2, 128, 512).transpose(2, 0, 1, 3))
    dev = {}
    dev["w_hp"] = bf(np.stack([pairs(fw["w_hp"][s], 8) for s in range(3)]))
    dev["w_r"] = pairs(fw["w_r"], 4)
    dev["w_m"] = bf(np.stack([pairs(fw["w_m"][s], 8) for s in range(3)]))
    dev["w_g"] = bf(np.stack([pairs(fw["w_g"][s], 8) for s in range(3)]))
    for s in range(3):
        dev[f"w_lp{s}"] = bf(fw["w_lp"][s])
    dev["w_f1all"] = bf(np.concatenate(
        [pairs(fw["w_f1l"][s], 2) for s in range(3)]
        + [pairs(fw["w_f1"][s], 4) for s in range(3)], axis=1))
    dev["negc_f1"] = bf(fw["negc_f1"][None])     # [1, 3, 512]
    dev["w_f2"] = pairs(fw["w_f2"], 4)
    # all per-partition bias/gamma vectors in one [128, ncols] tensor
    cols = []
    for name, nk, _ns in CVEC_SPEC:
        v = _vec_pp(fw[name], nk)
        v = v.reshape(128, -1) if v.ndim == 2 else \
            np.ascontiguousarray(v.transpose(1, 0, 2)).reshape(128, -1)
        assert v.shape[1] == CVEC_OFF[name][1], name
        cols.append(v)
    dev["cvec"] = np.ascontiguousarray(np.concatenate(cols, axis=1))
    dev["ones_col"] = np.ones((128, 1), BF_NP)
    dev["eps_lhs"] = np.full((1, 1), (D // 2) * EPS, BF_NP)
    dev["one_row"] = np.ones((1, 512), BF_NP)
    return dev


# --------------------------------------------------------------------------
# Device program
# --------------------------------------------------------------------------

def emit_program(tc, io):
    nc = tc.nc
    ACT = mybir.ActivationFunctionType
    ALU = mybir.AluOpType
    from contextlib import ExitStack
    ctx = ExitStack()

    P = lambda name, bufs, space="SBUF": ctx.enter_context(
        tc.tile_pool(name=name, bufs=bufs, space=space))
    const = P("const", 1)
    wpool = P("w", 3)
    xpool = P("x", 2)
    lpool = P("l", 6)
    big = P("big", 12)
    tpool = P("t", 2)
    mp = P("m", 8)
    evp = P("ev", 8)
    sqp = P("sq", 5)
    zp = P("z", 6)
    qp = P("q", 4)
    sgp = P("sg", 4)
    bcp = P("bc", 4)
    op_ = P("o", 4)
    rowf = P("rowf", 3)
    rowi = P("rowi", 3)
    wrp = P("wr", 6)
    ltp = P("lt", 4)
    f1p = P("f1w", 1)
    mm_ps = P("mm_ps", 6, "PSUM")
    st_ps = P("st_ps", 2, "PSUM")

    # ---------------- earliest DMAs: first stage inputs/weights ----------
    # (issued before the const loads so the Sync engine's serial trigger
    # stream starts the big startup transfers first)
    cvec = const.tile([128, CVEC_NCOLS], F32, name="cvec")
    nc.sync.dma_start(cvec[:], io["cvec"])
    x00 = xpool.tile([128, 4, 2, 512], BF, name="xc")
    nc.sync.dma_start(x00[:], io["xT0"][0])
    x01 = xpool.tile([128, 4, 2, 512], BF, name="xc")
    nc.sync.dma_start(x01[:], io["xT0"][1])
    w0 = wpool.tile([128, 4, 2, 512], BF, name="wc", tag="wc")
    nc.sync.dma_start(w0[:], io["w_hp"][0])

    # ---------------- constants ----------------
    def load(name, shape, rearr=None, dtype=F32):
        t = const.tile(shape, dtype, name=name)
        src = io[name]
        if rearr:
            src = src.rearrange(rearr)
        nc.sync.dma_start(t[:], src)
        return t

    ones_col = load("ones_col", [128, 1], dtype=BF)
    eps_lhs = load("eps_lhs", [1, 1], dtype=BF)
    one_row = load("one_row", [1, 512], dtype=BF)
    negc3 = load("negc_f1", [1, 3, 512], dtype=BF)

    def cv(name):
        off, n = CVEC_OFF[name]
        ap = cvec[:, off:off + n]
        if n > 4:
            ap = ap.rearrange("p (s c) -> p s c", s=3)
        return ap

    b_hp, b_m, g_hp, be_hp = cv("b_hp"), cv("b_m"), cv("g_hp"), cv("be_hp")
    g_n1, be_n1, b_g = cv("g_n1"), cv("be_n1"), cv("b_g")
    b_r, b_f1, b_f2 = cv("b_r"), cv("b_f1"), cv("b_f2")
    g_f1, be_f1, g_f2, be_f2 = cv("g_f1"), cv("be_f1"), cv("g_f2"), cv("be_f2")
    b_lp, g_lp, be_lp = cv("b_lp"), cv("g_lp"), cv("be_lp")
    w_lp = [load(f"w_lp{s}", [FS[s], 256], dtype=BF) for s in range(3)]

    # ---------------- helpers ----------------
    pend = []

    def flush(n=None):
        cnt = len(pend) if n is None else n
        for _ in range(cnt):
            if pend:
                pend.pop(0)()

    def load_pairs(dram_stage, npairs):
        """One DMA for a whole [128, npairs, 2, 512] weight stage."""
        wc = wpool.tile([128, npairs, 2, 512], BF, name="wc", tag="wc")
        nc.sync.dma_start(wc[:], dram_stage)
        fns = []
        for i in range(npairs):
            for cc in range(2):
                fns.append(lambda m, wc=wc, i=i, cc=cc:
                           wc[:, i, cc, ts(m, 128)])
        return fns

    def emit_mms(lhs_fns, rhs_fn, nm=4):
        ps = [mm_ps.tile([128, 512], F32, name="mm") for _ in range(nm)]
        last = len(lhs_fns) - 1
        for ci, lf in enumerate(lhs_fns):
            rhs = rhs_fn(ci)
            for m in range(nm):
                nc.tensor.matmul(ps[m][:], lf(m), rhs,
                                 start=(ci == 0), stop=(ci == last))
        return ps

    def evict_sq(ps_list, bias_cols, do_sq=True, pool=None, dve_half=True,
                 sq_ps=False):
        """Evict psum chunks (+bias) to bf16 SBUF; optionally square them.
        Evictions alternate ACT/DVE so the last chunk lands fast. With
        sq_ps, squares come straight from PSUM on ACT (Square is in every
        activation table, so no table reload)."""
        ev, sq = [], []
        for c, psx in enumerate(ps_list):
            e = (pool or evp).tile([128, 512], BF, name="evt")
            if dve_half and c % 2 == 1:
                nc.vector.tensor_scalar_add(e[:], psx[:], bias_cols[c])
            else:
                nc.scalar.activation(e[:], psx[:], ACT.Identity,
                                     bias=bias_cols[c])
            ev.append(e)
            if do_sq:
                s = sqp.tile([128, 512], BF, name="sqt")
                if sq_ps:
                    nc.scalar.activation(s[:], psx[:], ACT.Square,
                                         bias=bias_cols[c])
                else:
                    nc.vector.tensor_mul(s[:], e[:], e[:])
                sq.append(s)
        return ev, sq

    def stats(sq_list, add_eps=False):
        st = st_ps.tile([1, 512], F32, name="st", tag="stps")
        n = len(sq_list) + (1 if add_eps else 0)
        for c, s in enumerate(sq_list):
            nc.tensor.matmul(st[:], ones_col[:], s[:],
                             start=(c == 0), stop=(c == n - 1))
        if add_eps:
            nc.tensor.matmul(st[:], eps_lhs[:], one_row[:],
                             start=False, stop=True)
        return st

    def half_istd(st_ap, dim):
        """bf16 [1,512] row of 1/sqrt(st/dim), broadcast to 128 parts."""
        rec = rowf.tile([1, 512], F32, name="rec", tag="rowf")
        nc.vector.reciprocal_approx_fast(rec[:], st_ap)
        ist = rowi.tile([1, 512], BF, name="ist", tag="rowi")
        nc.scalar.activation(ist[:], rec[:], ACT.Sqrt, scale=float(dim))
        bc = bcp.tile([128, 512], BF, name="bct")
        nc.gpsimd.partition_broadcast(bc[:], ist[0:1, :])
        return bc

    def ln_finish(ev, st, dim, gam, bet, func, out_tile, via_ts=False):
        bc = half_istd(st[:], dim)
        for c, e in enumerate(ev):
            z = zp.tile([128, 512], BF, name="zzt")
            nc.vector.tensor_mul(z[:], e[:], bc[:])
            if via_ts:
                nc.vector.tensor_scalar(out_tile[:, c, :], z[:],
                                        gam[:, c:c + 1], bet[:, c:c + 1],
                                        ALU.mult, ALU.add)
            else:
                nc.scalar.activation(out_tile[:, c, :], z[:], func,
                                     bias=bet[:, c:c + 1],
                                     scale=gam[:, c:c + 1])

    # ---------------- per-stage state ----------------
    yh = [[None] * 2 for _ in range(3)]
    e_ = [[None] * 2 for _ in range(3)]
    m_ = [[None] * 2 for _ in range(3)]
    zt = [[None] * 2 for _ in range(3)]
    l_ = [[None] * 2 for _ in range(3)]
    h_ = [None] * 2
    wrow = [[None] * 3 for _ in range(2)]
    hpw = [None] * 3
    mw = [None] * 3
    gw = [None] * 3
    rw = [None]
    f1w = [None]
    f2w = [None]
    hpx = [[None] * 2 for _ in range(3)]

    def lp_group(s, bt):
        bsl = ts(bt, 512)
        lt = ltp.tile([FS[s], 512], F32, name="ltt", tag="ltt")
        nc.sync.dma_start(lt[:], io[f"lT{s}"][:, bsl])
        lsg = ltp.tile([FS[s], 512], BF, name="lsg", tag="ltt")
        nc.scalar.activation(lsg[:], lt[:], ACT.Sigmoid)
        ps = [mm_ps.tile([128, 512], F32, name="mm") for _ in range(2)]
        for m in range(2):
            nc.tensor.matmul(ps[m][:], w_lp[s][:, ts(m, 128)], lsg[:],
                             start=True, stop=True)
        ev, sq = evict_sq(ps, [b_lp[:, s, c:c + 1] for c in range(2)])

        def fin(s=s, bt=bt, ev=ev, sq=sq):
            st = stats(sq, add_eps=True)
            l_sb = lpool.tile([128, 2, 512], BF, name="l_sb")
            ln_finish(ev, st, D // 2, g_lp[:, s], be_lp[:, s], ACT.Gelu,
                      l_sb)
            l_[s][bt] = l_sb
        pend.append(fin)

    def hp_prefetch(s, bt):
        if s == 0:
            hpx[0] = [x00, x01]
            if bt == 0:
                fns = []
                for i in range(4):
                    for cc in range(2):
                        fns.append(lambda m, i=i, cc=cc:
                                   w0[:, i, cc, ts(m, 128)])
                hpw[0] = fns
            return
        xc = xpool.tile([128, 4, 2, 512], BF, name="xc")
        nc.sync.dma_start(xc[:], io[f"xT{s}"][bt])
        hpx[s][bt] = xc
        if bt == 0:
            hpw[s] = load_pairs(io["w_hp"][s], 4)

    def hp_group(s, bt):
        xc = hpx[s][bt]
        ps = emit_mms(hpw[s], lambda c: xc[:, c // 2, c % 2, :])
        ev, sq = evict_sq(ps, [b_hp[:, s, c:c + 1] for c in range(4)])

        def fin(s=s, bt=bt, ev=ev, sq=sq):
            st = stats(sq)
            y_sb = big.tile([128, 4, 512], BF, name="big_sb")
            ln_finish(ev, st, D, g_hp[:, s], be_hp[:, s], ACT.Gelu, y_sb)
            yh[s][bt] = y_sb
        pend.append(fin)

    def r_group(s, bt):
        if rw[0] is None:
            rw[0] = load_pairs(io["w_r"], 2)
        ps = emit_mms(rw[0], lambda c: yh[s][bt][:, c, :])
        ev, sq = evict_sq(ps, [b_r[:, c:c + 1] for c in range(4)])

        def fin(s=s, bt=bt, ev=ev, sq=sq):
            st = stats(sq)
            bc = half_istd(st[:], D)
            e_sb = big.tile([128, 4, 512], BF, name="big_sb")
            for c, et in enumerate(ev):
                nc.vector.scalar_tensor_tensor(e_sb[:, c, :], et[:],
                                               g_n1[:, s, c:c + 1], bc[:],
                                               ALU.mult, ALU.mult)
            e_[s][bt] = e_sb
        pend.append(fin)

    m_streams = [(1, 2), (0, 2), (0, 1)]

    def m_group(s, bt):
        if bt == 0:
            mw[s] = load_pairs(io["w_m"][s], 4)
        sa, sb = m_streams[s]
        ps = emit_mms(mw[s], lambda c: (e_[sa][bt][:, c, :] if c < 4
                                        else e_[sb][bt][:, c - 4, :]))
        ev, _ = evict_sq(ps, [b_m[:, s, c:c + 1] for c in range(4)],
                         do_sq=False, pool=mp)
        m_[s][bt] = ev

    def g_group(s, bt):
        if bt == 0:
            gw[s] = load_pairs(io["w_g"][s], 4)
        ps = emit_mms(gw[s], lambda c: (e_[s][bt][:, c, :] if c < 4
                                        else m_[s][bt][c - 4][:]))
        t_sb = tpool.tile([128, 4, 512], BF, name="t_sb")
        sqs = []
        for c in range(4):
            sg = sgp.tile([128, 512], BF, name="sgt")
            nc.scalar.activation(sg[:], ps[c][:], ACT.Sigmoid,
                                 bias=b_g[:, s, c:c + 1])
            q = qp.tile([128, 512], BF, name="qt")
            nc.vector.tensor_mul(q[:], sg[:], m_[s][bt][c][:])
            nc.vector.scalar_tensor_tensor(t_sb[:, c, :],
                                           e_[s][bt][:, c, :],
                                           be_n1[:, s, c:c + 1], q[:],
                                           ALU.add, ALU.add)
            sqc = sqp.tile([128, 512], BF, name="sqt")
            nc.vector.tensor_mul(sqc[:], t_sb[:, c, :], t_sb[:, c, :])
            sqs.append(sqc)

        def fin(s=s, bt=bt, t_sb=t_sb, sqs=sqs):
            st_sum = st_ps.tile([1, 512], F32, name="st", tag="stps")
            for c in range(4):
                nc.tensor.matmul(st_sum[:], ones_col[:], t_sb[:, c, :],
                                 start=(c == 0), stop=(c == 3))
            st_sq = stats(sqs)
            mu = rowf.tile([1, 512], F32, name="mu", tag="rowf")
            nc.scalar.activation(mu[:], st_sum[:], ACT.Copy, scale=1.0 / D)
            v = rowf.tile([1, 512], F32, name="vv", tag="rowf")
            m2 = rowf.tile([1, 512], F32, name="m2", tag="rowf")
            nc.vector.tensor_mul(m2[:], mu[:], mu[:])
            nc.vector.scalar_tensor_tensor(v[:], m2[:], -float(D), st_sq[:],
                                           ALU.mult, ALU.add)
            rec = rowf.tile([1, 512], F32, name="rec", tag="rowf")
            nc.vector.reciprocal_approx_fast(rec[:], v[:])
            ist = rowi.tile([1, 512], BF, name="ist", tag="rowi")
            nc.scalar.activation(ist[:], rec[:], ACT.Sqrt, scale=float(D))
            wr = wrp.tile([1, 512], BF, name="wr1")
            nc.vector.tensor_mul(wr[:], mu[:], ist[:])
            wrow[bt][s] = wr
            bc = bcp.tile([128, 512], BF, name="bct")
            nc.gpsimd.partition_broadcast(bc[:], ist[0:1, :])
            zt_sb = big.tile([128, 4, 512], BF, name="big_sb")
            for c in range(4):
                nc.vector.tensor_mul(zt_sb[:, c, :], t_sb[:, c, :], bc[:])
            zt[s][bt] = zt_sb
        pend.append(fin)

    def f1_group(bt):
        if bt == 0:
            wc = f1p.tile([128, 9, 2, 512], BF, name="f1wc")
            nc.sync.dma_start(wc[:], io["w_f1all"])
            mk = lambda pi, cc: (lambda m, pi=pi, cc=cc:
                                 wc[:, pi, cc, ts(m, 128)])
            f1lw = [[mk(s, c) for c in range(2)] for s in range(3)]
            f1ww = [[mk(3 + 2 * s + c // 2, c % 2) for c in range(4)]
                    for s in range(3)]
            f1w[0] = (f1lw, f1ww)
        f1lw, f1ww = f1w[0]
        ps = [mm_ps.tile([128, 512], F32, name="mm") for _ in range(4)]
        seq = []
        for s in range(3):
            seq += [(f1lw[s][c], l_[s][bt][:, c, :]) for c in range(2)]
        for s in (2, 1, 0):
            seq += [(f1ww[s][c], zt[s][bt][:, c, :]) for c in range(4)]
        for s in (2, 1, 0):
            seq.append((lambda m, s=s: negc3[0:1, s, ts(m, 128)],
                        wrow[bt][s][:]))
        last = len(seq) - 1
        for ci, (lf, rhs) in enumerate(seq):
            for m in range(4):
                nc.tensor.matmul(ps[m][:], lf(m), rhs,
                                 start=(ci == 0), stop=(ci == last))
        ev, sq = evict_sq(ps, [b_f1[:, c:c + 1] for c in range(4)])

        def fin(bt=bt, ev=ev, sq=sq):
            st = stats(sq)
            h_sb = big.tile([128, 4, 512], BF, name="big_sb")
            ln_finish(ev, st, D, g_f1, be_f1, ACT.Gelu, h_sb)
            h_[bt] = h_sb
        pend.append(fin)

    def f2_group(bt):
        bsl = ts(bt, 512)
        if bt == 0:
            f2w[0] = load_pairs(io["w_f2"], 2)
        ps = emit_mms(f2w[0], lambda c: h_[bt][:, c, :])
        ev, sq = evict_sq(ps, [b_f2[:, c:c + 1] for c in range(4)])

        def fin(bt=bt, bsl=bsl, ev=ev, sq=sq):
            st = stats(sq)
            bc = half_istd(st[:], D)
            for c, et in enumerate(ev):
                z = op_.tile([128, 512], F32, name="ot")
                nc.vector.tensor_mul(z[:], et[:], bc[:])
                o = op_.tile([128, 512], F32, name="ot")
                nc.vector.tensor_scalar(o[:], z[:], g_f2[:, c:c + 1],
                                        be_f2[:, c:c + 1], ALU.mult, ALU.add)
                nc.sync.dma_start(io["outT"][ts(c, 128), bsl], o[:])
        pend.append(fin)

    # ---------------- emission schedule ----------------
    # One-behind-ish flushing: fin(G) is flushed ~2 steps after G, always
    # >=2 steps before G's consumer. PSUM banks are freed by the inline
    # evictions, so delayed fins never gate bank reuse.
    hp_prefetch(0, 0); hp_prefetch(0, 1)
    hp_group(0, 0)
    hp_prefetch(1, 0)
    hp_group(0, 1)
    hp_prefetch(1, 1)
    hp_group(1, 0)
    hp_group(1, 1); flush(1)      # hp00
    hp_prefetch(2, 0); hp_prefetch(2, 1)
    hp_group(2, 0); flush(1)      # hp01
    hp_group(2, 1); flush(1)      # hp10
    r_group(0, 0); flush(1)       # hp11
    lp_group(0, 0)
    r_group(1, 0); flush(1)       # hp20
    lp_group(0, 1)
    r_group(2, 0); flush(1)       # hp21
    lp_group(1, 0)
    r_group(0, 1); flush(2)       # r00 -> e0b0, lp00
    lp_group(1, 1)
    r_group(1, 1); flush(2)       # r10 -> e1b0, lp01
    lp_group(2, 0)
    m_group(2, 0); flush(2)       # r20 -> e2b0, lp10
    lp_group(2, 1)
    r_group(2, 1); flush(2)       # r01 -> e0b1, lp11
    flush(2)                      # r11 -> e1b1, lp20
    m_group(2, 1)
    flush(2)                      # r21 -> e2b1, lp21
    g_group(2, 0)
    g_group(2, 1)
    m_group(1, 0)
    m_group(1, 1); flush(1)       # g20 -> zt2b0
    g_group(1, 0); flush(1)       # g21 -> zt2b1
    g_group(1, 1)
    m_group(0, 0)
    m_group(0, 1); flush(1)       # g10 -> zt1b0
    g_group(0, 0); flush(1)       # g11 -> zt1b1
    g_group(0, 1); flush(1)       # g00 -> zt0b0
    f1_group(0); flush(2)         # g01 -> zt0b1, f1(0) -> h0
    f1_group(1); flush(1)         # f1(1) -> h1
    f2_group(0); flush(1)         # f2(0) -> out b0
    f2_group(1)
    flush()
    ctx.close()


def build_program():
    nc = bacc.Bacc("TRN2", target_bir_lowering=False, debug=False,
                   num_devices=NCORES)
    io = {}

    def din(name, shape, dtype=F32):
        io[name] = nc.dram_tensor(name, list(shape), dtype,
                                  kind="ExternalInput").ap()

    for s in range(3):
        din(f"xT{s}", (2, 128, 4, 2, 512), dtype=BF)
        din(f"lT{s}", (FS[s], BL))
    din("w_hp", (3, 128, 4, 2, 512), dtype=BF)
    din("w_r", (128, 2, 2, 512), dtype=BF)
    din("w_m", (3, 128, 4, 2, 512), dtype=BF)
    din("w_g", (3, 128, 4, 2, 512), dtype=BF)
    for s in range(3):
        din(f"w_lp{s}", (FS[s], 256), dtype=BF)
    din("w_f1all", (128, 9, 2, 512), dtype=BF)
    din("negc_f1", (1, 3, 512), dtype=BF)
    din("w_f2", (128, 2, 2, 512), dtype=BF)
    din("cvec", (128, CVEC_NCOLS))
    din("ones_col", (128, 1), dtype=BF)
    din("eps_lhs", (1, 1), dtype=BF)
    din("one_row", (1, 512), dtype=BF)
    io["outT"] = nc.dram_tensor("outT", [D, BL], F32,
                                kind="ExternalOutput").ap()

    with tile.TileContext(nc) as tc:
        emit_program(tc, io)
    nc.compile()
    return nc


def make_in_maps(inputs):
    fw = fold_weights(inputs)
    dev = device_arrays(fw)
    hidden = [np.asarray(inputs["verb_hidden"], np.float32),
              np.asarray(inputs["inst_hidden"], np.float32),
              np.asarray(inputs["target_hidden"], np.float32)]
    logits = [np.asarray(inputs["verb_logits"], np.float32),
              np.asarray(inputs["inst_logits"], np.float32),
              np.asarray(inputs["target_logits"], np.float32)]
    in_maps = []
    for core in range(NCORES):
        rows = slice(core * BL, (core + 1) * BL)
        m = dict(dev)
        for s in range(3):
            xm = hidden[s][rows].T.reshape(4, 2, 128, 2, 512)
            m[f"xT{s}"] = np.ascontiguousarray(
                xm.transpose(3, 2, 0, 1, 4)).astype(BF_NP)
            m[f"lT{s}"] = np.ascontiguousarray(logits[s][rows].T)
        in_maps.append(m)
    return in_maps


_NC_CACHE = None


def _run(inputs, **spmd_kwargs):
    global _NC_CACHE
    if _NC_CACHE is None:
        _NC_CACHE = build_program()
    nc = _NC_CACHE
    in_maps = make_in_maps(inputs)
    res = run_bass_kernel_spmd(nc, in_maps, list(range(NCORES)),
                               **spmd_kwargs)
    out = np.empty((B, D), dtype=np.float32)
    for core in range(NCORES):
        out[core * BL:(core + 1) * BL] = res.results[core]["outT"].T
    return out, res


def kernel(**inputs) -> np.ndarray:
    return _run(inputs)[0]


def kernel_profiled(inputs, tmpdir=None):
    """Returns (out, BassKernelResults) with an NTFF-based profile."""
    return _run(inputs, trace=True, tmpdir=tmpdir)
